# revision 32
# baseline (speedup 1.0000x reference)
"""EnsembleRSSM observe kernel for 8 Trainium2 NeuronCores.

Strategy: data-parallel over batch (B=128 -> 16 rows/core), zero cross-core
communication. The sequential T=64 scan runs per-core on its batch shard with
all scan weights resident in SBUF (bf16 matmul operands, fp32 PSUM/vector
math).

v2: "folded" layout — every [16, F] activation is stored [128, F/4] with
batch in the low 16 rows of each 32-partition strip and feature chunks
spread over 4 strips x column groups (chunk c -> strip c%4, colgroup c//4).
Matmuls use tile_position column tiling (4 concurrent streams -> ~4x PE
throughput at M=16), vector/scalar elementwise ops run at full 128-partition
occupancy, PE transposes shrink to [128,128] blocks (one per colgroup),
LayerNorm strip-reduction+broadcast is one constant "selrep" matmul, rsqrt is
a DVE bit-trick + Newton (no act-table switches: the whole scan runs on the
exp table with sigmoid written as tanh), and softplus is deferred to a
batched post-pass.  The obs-dist head is computed feature-major so the stoch
recurrence needs no extra transpose; its transposed output layout is fixed up
on the host.
"""
import sys

sys.path.insert(0, "/opt/trn_rl_repo")

import numpy as np
import ml_dtypes

import concourse.bass as bass
import concourse.bacc as bacc
import concourse.mybir as mybir
import concourse.tile as tile
from concourse import bass_utils

_orig_get_tables = bacc.get_activation_tables

def _filtered_tables(arch):
    tabs = _orig_get_tables(arch)
    keep = ("exp_and_others",)
    if not all(k in tabs for k in keep):
        return tabs
    return {k: (v if k in keep else set()) for k, v in tabs.items()}

bacc.get_activation_tables = _filtered_tables

B, T = 128, 64
EMB, ACT = 1536, 12
STOCH, DETER, HIDDEN, ENS = 32, 1024, 1024, 5
MIN_STD = 0.1
UPDATE_BIAS = -1.0
NC = 8
BL = B // NC  # 16 rows per core

F32 = mybir.dt.float32
BF16 = mybir.dt.bfloat16
I32 = mybir.dt.int32
AF = mybir.ActivationFunctionType
ALU = mybir.AluOpType
AX = mybir.AxisListType.X

MAGIC = 0x5F3759DF
# seed magics for rsqrt computed from W*q (SBUF raw sums) instead of q:
# rsqrt(q) with input W*q needs magic + log2(W)*2^22
MAGIC_G = MAGIC + int(round((np.log2(3 * 1024)) * (1 << 22)))
MAGIC_H = MAGIC + int(round((np.log2(1024)) * (1 << 22)))
NEWTON_ITERS = 1

_CACHE = {}
_LAST_IN_MAPS = None


def _row_layout(ens_index):
    """Head-sorted, 128-padded row layout for phase 2."""
    order = {}
    for t in range(T):
        order.setdefault(int(ens_index[t]), []).append(t)
    row_ofs = [0] * T
    heads = []
    r = 0
    for k in range(ENS):
        ts = order.get(k, [])
        r0 = r
        for t in ts:
            row_ofs[t] = r
            r += BL
        r = ((r + 127) // 128) * 128
        heads.append((k, r0, r, ts))
    return row_ofs, heads, max(r, 128)


def _build(ens_index, affine_trivial):
    nc = bacc.Bacc("TRN2", target_bir_lowering=False, debug=False,
                   num_devices=NC, detect_race_conditions=False)
    for cval in sorted({-1.0, -0.5, 0.5, 1.0, 1e-5, 0.5 * UPDATE_BIAS, MIN_STD}):
        th = nc.alloc_sbuf_tensor(f"constx-{cval}", [128, 1], F32)
        nc.gpsimd.memset(th.ap(), cval)
        nc.const_aps.aps[(F32, cval)] = th.ap()

    # ---- DRAM inputs ----
    d_embT = nc.dram_tensor("embT", [EMB, T * BL], BF16, kind="ExternalInput")
    d_We = nc.dram_tensor("We", [EMB, HIDDEN], BF16, kind="ExternalInput")
    d_Wg = nc.dram_tensor("Wg", [128, 16 * 8 * 385], BF16, kind="ExternalInput")
    d_Wd = nc.dram_tensor("Wd", [128, 8 * 4 * 256], BF16, kind="ExternalInput")
    d_Wimg = nc.dram_tensor("Wimg", [45, 4 * 257], BF16, kind="ExternalInput")
    d_Wod = nc.dram_tensor("Wod", [128, 8 * 64], BF16, kind="ExternalInput")
    d_Wio = nc.dram_tensor("Wio", [ENS, DETER, HIDDEN], BF16, kind="ExternalInput")
    d_Wids = nc.dram_tensor("Wids", [ENS, HIDDEN, 2 * STOCH], BF16, kind="ExternalInput")
    d_maskF = nc.dram_tensor("maskF", [128, T + 1], F32, kind="ExternalInput")
    d_maskTb = nc.dram_tensor("maskTb", [STOCH, (T + 1) * BL], F32, kind="ExternalInput")
    d_am1T = nc.dram_tensor("am1T", [ACT + 1, T * BL], BF16, kind="ExternalInput")
    d_selg = nc.dram_tensor("selg", [128, 128], F32, kind="ExternalInput")
    d_selh = nc.dram_tensor("selh", [128, 128], F32, kind="ExternalInput")
    d_eye128b = nc.dram_tensor("eye128b", [128, 128], BF16, kind="ExternalInput")
    d_eye128 = nc.dram_tensor("eye128", [128, 128], F32, kind="ExternalInput")
    # non-trivial affine support
    d_gbn_g = nc.dram_tensor("gbn_gru", [128, 4 * 384], F32, kind="ExternalInput")
    d_gbn_o = nc.dram_tensor("gbn_obs", [128, 2 * 256], F32, kind="ExternalInput")
    d_gbn_i = nc.dram_tensor("gbn_img", [128, 2 * 256], F32, kind="ExternalInput")
    d_bod = nc.dram_tensor("b_od", [2 * STOCH, BL], F32, kind="ExternalInput")
    d_gbn_e = nc.dram_tensor("gbn_ens", [ENS, 128, 2 * HIDDEN], F32, kind="ExternalInput")
    d_bias_o = nc.dram_tensor("bias_obs", [128, HIDDEN], F32, kind="ExternalInput")
    d_bias_e = nc.dram_tensor("bias_ens", [ENS, 128, HIDDEN], F32, kind="ExternalInput")
    d_bids = nc.dram_tensor("b_ids", [ENS, 128, 2 * STOCH], F32, kind="ExternalInput")

    o_detF = nc.dram_tensor("o_detF", [128, T, 256], F32, kind="ExternalOutput")
    o_meanT = nc.dram_tensor("o_meanT", [STOCH, T, BL], F32, kind="ExternalOutput")
    o_rawsT = nc.dram_tensor("o_rawsT", [STOCH, T, BL], F32, kind="ExternalOutput")
    o_ens = nc.dram_tensor("o_ens", [BL, T, 2 * STOCH], F32, kind="ExternalOutput")

    row_ofs, heads, R = _row_layout(ens_index)
    s_xop = nc.dram_tensor("xo_pre", [T * BL, HIDDEN], F32)     # column-permuted
    s_dTa = nc.dram_tensor("deterT_all", [8, 128, R], BF16)
    # phase-2 groups become ready when their last timestep's deterT lands
    group_ready = {}
    gts_by_group = {}
    for (hk, r0, r1, ts) in heads:
        for mt in range(r0 // 128, r1 // 128):
            i0 = (mt * 128 - r0) // BL
            gts = ts[i0:i0 + 8]
            if gts:
                group_ready.setdefault(max(gts), []).append((hk, mt))
                gts_by_group[(hk, mt)] = gts

    KT_E = EMB // 128  # 12

    with tile.TileContext(nc) as tc:
        with tc.tile_pool(name="wpool", bufs=1) as wpool, \
             tc.tile_pool(name="spool", bufs=1) as spool, \
             tc.tile_pool(name="spool2", bufs=2) as spool2:

            # ---------- resident weights ----------
            Wg = wpool.tile([128, 16 * 8 * 385], BF16, tag="Wg")
            for k in range(16):
                nc.sync.dma_start(Wg[:, k * 3080:(k + 1) * 3080],
                                  d_Wg.ap()[:, k * 3080:(k + 1) * 3080])
            Wd = wpool.tile([128, 8 * 4 * 256], BF16, tag="Wd")
            nc.sync.dma_start(Wd[:], d_Wd.ap())
            Wimg = wpool.tile([45, 4 * 257], BF16, tag="Wimg")
            nc.sync.dma_start(Wimg[:], d_Wimg.ap())
            Wod = wpool.tile([128, 8 * 64], BF16, tag="Wod")
            nc.sync.dma_start(Wod[:], d_Wod.ap())
            selg = wpool.tile([128, 128], F32, tag="selg")
            nc.sync.dma_start(selg[:], d_selg.ap())
            selh = wpool.tile([128, 128], F32, tag="selh")
            nc.sync.dma_start(selh[:], d_selh.ap())
            eye128b = wpool.tile([128, 128], BF16, tag="eye128b")
            nc.sync.dma_start(eye128b[:], d_eye128b.ap())
            eye128 = wpool.tile([128, 128], F32, tag="eye128")
            nc.sync.dma_start(eye128[:], d_eye128.ap())
            maskF = wpool.tile([128, T + 1], F32, tag="maskF")
            nc.sync.dma_start(maskF[:], d_maskF.ap())
            maskTb = wpool.tile([STOCH, (T + 1) * BL], F32, tag="maskTb")
            nc.sync.dma_start(maskTb[:], d_maskTb.ap())
            magic = wpool.tile([128, 1], I32, tag="magic")
            nc.vector.memset(magic[:], MAGIC)
            magic_g = wpool.tile([128, 1], I32, tag="magic_g")
            nc.vector.memset(magic_g[:], MAGIC_G)
            magic_h = wpool.tile([128, 1], I32, tag="magic_h")
            nc.vector.memset(magic_h[:], MAGIC_H)
            if not affine_trivial:
                gbn_g = wpool.tile([128, 4 * 384], F32, tag="gg")
                nc.sync.dma_start(gbn_g[:], d_gbn_g.ap())
                gbn_o = wpool.tile([128, 2 * 256], F32, tag="go")
                nc.sync.dma_start(gbn_o[:], d_gbn_o.ap())
                gbn_i = wpool.tile([128, 2 * 256], F32, tag="gi")
                nc.sync.dma_start(gbn_i[:], d_gbn_i.ap())
                bod = wpool.tile([2 * STOCH, BL], F32, tag="bod")
                nc.sync.dma_start(bod[:], d_bod.ap())

            # ---------- phase 0: xop = embed @ We (+bias), column-permuted ----------
            with tc.tile_pool(name="p0w", bufs=1) as p0w, \
                 tc.tile_pool(name="p0e", bufs=13) as p0e, \
                 tc.tile_pool(name="p0p", bufs=2, space="PSUM") as p0p:
                bias_o_sb = None
                if not affine_trivial:
                    bias_o_sb = p0w.tile([128, HIDDEN], F32, tag="biaso")
                    nc.sync.dma_start(bias_o_sb[:], d_bias_o.ap())
                wek = []
                for k in range(KT_E):
                    wt = p0w.tile([128, HIDDEN], BF16, tag=f"p0we{k}", name=f"p0we{k}")
                    nc.sync.dma_start(wt[:], d_We.ap()[k * 128:(k + 1) * 128, :])
                    wek.append(wt)
                for mt in range(T * BL // 128):
                    ek = []
                    for k in range(KT_E):
                        ekt = p0e.tile([128, 128], BF16, tag="p0ek")
                        nc.sync.dma_start(ekt[:], d_embT.ap()[k * 128:(k + 1) * 128,
                                                              mt * 128:(mt + 1) * 128])
                        ek.append(ekt)
                    xop0 = spool.tile([128, HIDDEN], F32, tag="p0out")
                    for nt in range(2):
                        ps = p0p.tile([128, 512], F32, tag="p0ps")
                        for k in range(KT_E):
                            nc.tensor.matmul(
                                ps[:], ek[k][:],
                                wek[k][:, nt * 512:(nt + 1) * 512],
                                start=(k == 0), stop=(k == KT_E - 1))
                        # chunk c = 4*nt + j  ->  permuted col j*256 + nt*128
                        for j in range(4):
                            dst = xop0[:, j * 256 + nt * 128:j * 256 + nt * 128 + 128]
                            src = ps[:, j * 128:(j + 1) * 128]
                            if bias_o_sb is not None:
                                nc.vector.tensor_add(
                                    dst, src,
                                    bias_o_sb[:, j * 256 + nt * 128:j * 256 + nt * 128 + 128])
                            elif j % 2 == 0:
                                nc.scalar.copy(dst, src)
                            else:
                                nc.vector.tensor_copy(dst, src)
                    nc.sync.dma_start(s_xop.ap()[mt * 128:(mt + 1) * 128, :], xop0[:])

                # zero the pad regions of deterT_all
                zpad = spool.tile([128, 128], BF16, tag="zpad")
                nc.vector.memset(zpad[:], 0.0)
                for (_k, r0, r1, ts) in heads:
                    pr0 = r0 + len(ts) * BL
                    if pr0 < r1:
                        for kk in range(8):
                            nc.sync.dma_start(s_dTa.ap()[kk, :, pr0:r1], zpad[:, 0:r1 - pr0])

            # ---------- scan ----------
            with tc.tile_pool(name="gpool", bufs=2, space="PSUM") as gpool, \
                 tc.tile_pool(name="xpool", bufs=1, space="PSUM") as xpool, \
                 tc.tile_pool(name="opool", bufs=1, space="PSUM") as opool, \
                 tc.tile_pool(name="dpool", bufs=1, space="PSUM") as dpool, \
                 tc.tile_pool(name="tpool", bufs=2, space="PSUM") as tpool, \
                 tc.tile_pool(name="p2p", bufs=1, space="PSUM") as p2p, \
                 tc.tile_pool(name="p2w", bufs=2) as p2w, \
                 tc.tile_pool(name="p2d", bufs=10) as p2d:

                def rsqrt_dve(st, ps_tile, pcol, outcol, qsb, magic_w):
                    """copy-based m/q; m^2 via vector TT (known good)."""
                    nc.vector.tensor_copy(st[:, 4:6], ps_tile[:, pcol:pcol + 2])
                    m, q = st[:, 4:5], st[:, 5:6]
                    nc.vector.tensor_tensor(st[:, 1:2], m, m, ALU.mult)
                    v = st[:, 0:1]
                    nc.vector.scalar_tensor_tensor(v, q, 1e-5, st[:, 1:2],
                                                   ALU.add, ALU.subtract)
                    yi = st[:, outcol:outcol + 1]
                    nc.vector.tensor_scalar(st[:, 2:3].bitcast(I32), v.bitcast(I32),
                                            1, None, ALU.arith_shift_right)
                    nc.vector.scalar_tensor_tensor(yi.bitcast(I32), magic[:], 1,
                                                   st[:, 2:3].bitcast(I32),
                                                   ALU.mult, ALU.subtract)
                    h = st[:, 1:2]
                    for _ in range(NEWTON_ITERS):
                        nc.vector.tensor_tensor(h, yi, yi, ALU.mult)
                        nc.vector.tensor_tensor(h, h, v, ALU.mult)
                        nc.vector.tensor_scalar(h, h, -0.5, 1.5, ALU.mult, ALU.add)
                        nc.vector.tensor_tensor(yi, yi, h, ALU.mult)
                    return m, yi

                magic = wpool.tile([128, 1], I32, tag="magic")
            nc.vector.memset(magic[:], MAGIC)
            magic_g = wpool.tile([128, 1], I32, tag="magic_g")
            nc.vector.memset(magic_g[:], MAGIC_G)
            magic_h = wpool.tile([128, 1], I32, tag="magic_h")
            nc.vector.memset(magic_h[:], MAGIC_H)
            if not affine_trivial:
                gbn_g = wpool.tile([128, 4 * 384], F32, tag="gg")
                nc.sync.dma_start(gbn_g[:], d_gbn_g.ap())
                gbn_o = wpool.tile([128, 2 * 256], F32, tag="go")
                nc.sync.dma_start(gbn_o[:], d_gbn_o.ap())
                gbn_i = wpool.tile([128, 2 * 256], F32, tag="gi")
                nc.sync.dma_start(gbn_i[:], d_gbn_i.ap())
                bod = wpool.tile([2 * STOCH, BL], F32, tag="bod")
                nc.sync.dma_start(bod[:], d_bod.ap())

            # ---------- phase 0: xop = embed @ We (+bias), column-permuted ----------
            with tc.tile_pool(name="p0w", bufs=1) as p0w, \
                 tc.tile_pool(name="p0e", bufs=13) as p0e, \
                 tc.tile_pool(name="p0p", bufs=2, space="PSUM") as p0p:
                bias_o_sb = None
                if not affine_trivial:
                    bias_o_sb = p0w.tile([128, HIDDEN], F32, tag="biaso")
                    nc.sync.dma_start(bias_o_sb[:], d_bias_o.ap())
                wek = []
                for k in range(KT_E):
                    wt = p0w.tile([128, HIDDEN], BF16, tag=f"p0we{k}", name=f"p0we{k}")
                    nc.sync.dma_start(wt[:], d_We.ap()[k * 128:(k + 1) * 128, :])
                    wek.append(wt)
                for mt in range(T * BL // 128):
                    ek = []
                    for k in range(KT_E):
                        ekt = p0e.tile([128, 128], BF16, tag="p0ek")
                        nc.sync.dma_start(ekt[:], d_embT.ap()[k * 128:(k + 1) * 128,
                                                              mt * 128:(mt + 1) * 128])
                        ek.append(ekt)
                    xop0 = spool.tile([128, HIDDEN], F32, tag="p0out")
                    for nt in range(2):
                        ps = p0p.tile([128, 512], F32, tag="p0ps")
                        for k in range(KT_E):
                            nc.tensor.matmul(
                                ps[:], ek[k][:],
                                wek[k][:, nt * 512:(nt + 1) * 512],
                                start=(k == 0), stop=(k == KT_E - 1))
                        # chunk c = 4*nt + j  ->  permuted col j*256 + nt*128
                        for j in range(4):
                            dst = xop0[:, j * 256 + nt * 128:j * 256 + nt * 128 + 128]
                            src = ps[:, j * 128:(j + 1) * 128]
                            if bias_o_sb is not None:
                                nc.vector.tensor_add(
                                    dst, src,
                                    bias_o_sb[:, j * 256 + nt * 128:j * 256 + nt * 128 + 128])
                            elif j % 2 == 0:
                                nc.scalar.copy(dst, src)
                            else:
                                nc.vector.tensor_copy(dst, src)
                    nc.sync.dma_start(s_xop.ap()[mt * 128:(mt + 1) * 128, :], xop0[:])

                # zero the pad regions of deterT_all
                zpad = spool.tile([128, 128], BF16, tag="zpad")
                nc.vector.memset(zpad[:], 0.0)
                for (_k, r0, r1, ts) in heads:
                    pr0 = r0 + len(ts) * BL
                    if pr0 < r1:
                        for kk in range(8):
                            nc.sync.dma_start(s_dTa.ap()[kk, :, pr0:r1], zpad[:, 0:r1 - pr0])

            # ---------- scan ----------
            with tc.tile_pool(name="gpool", bufs=2, space="PSUM") as gpool, \
                 tc.tile_pool(name="xpool", bufs=1, space="PSUM") as xpool, \
                 tc.tile_pool(name="opool", bufs=1, space="PSUM") as opool, \
                 tc.tile_pool(name="dpool", bufs=1, space="PSUM") as dpool, \
                 tc.tile_pool(name="tpool", bufs=2, space="PSUM") as tpool, \
                 tc.tile_pool(name="p2p", bufs=1, space="PSUM") as p2p, \
                 tc.tile_pool(name="p2w", bufs=2) as p2w, \
                 tc.tile_pool(name="p2d", bufs=10) as p2d:

                def rsqrt_dve(st, ps_tile, pcol, outcol, qsb, magic_w):
                    """st[:, outcol] = 1/sqrt(q - m^2 + eps), (m, q) = psum tile
                    cols (pcol, pcol+1). m^2 on scalar (psum read) in parallel;
                    v in sbuf; bit-trick seed + Newton. Returns (m_psum, rstd)."""
                    m = ps_tile[:, pcol:pcol + 1]
                    q = ps_tile[:, pcol + 1:pcol + 2]
                    nc.scalar.square(st[:, 1:2], m)
                    v = st[:, 0:1]
                    nc.vector.scalar_tensor_tensor(v, q, 1e-5, st[:, 1:2],
                                                   ALU.add, ALU.subtract)
                    yi = st[:, outcol:outcol + 1]
                    nc.vector.tensor_scalar(st[:, 2:3].bitcast(I32), v.bitcast(I32),
                                            1, None, ALU.arith_shift_right)
                    nc.vector.scalar_tensor_tensor(yi.bitcast(I32), magic[:], 1,
                                                   st[:, 2:3].bitcast(I32),
                                                   ALU.mult, ALU.subtract)
                    h = st[:, 1:2]
                    for _ in range(NEWTON_ITERS):
                        nc.vector.tensor_tensor(h, yi, yi, ALU.mult)
                        nc.vector.tensor_tensor(h, h, v, ALU.mult)
                        nc.vector.tensor_scalar(h, h, -0.5, 1.5, ALU.mult, ALU.add)
                        nc.vector.tensor_tensor(yi, yi, h, ALU.mult)
                    return m, yi

                magic = wpool.tile([128, 1], I32, tag="magic")
            nc.vector.memset(magic[:], MAGIC)
            magic_g = wpool.tile([128, 1], I32, tag="magic_g")
            nc.vector.memset(magic_g[:], MAGIC_G)
            magic_h = wpool.tile([128, 1], I32, tag="magic_h")
            nc.vector.memset(magic_h[:], MAGIC_H)
            if not affine_trivial:
                gbn_g = wpool.tile([128, 4 * 384], F32, tag="gg")
                nc.sync.dma_start(gbn_g[:], d_gbn_g.ap())
                gbn_o = wpool.tile([128, 2 * 256], F32, tag="go")
                nc.sync.dma_start(gbn_o[:], d_gbn_o.ap())
                gbn_i = wpool.tile([128, 2 * 256], F32, tag="gi")
                nc.sync.dma_start(gbn_i[:], d_gbn_i.ap())
                bod = wpool.tile([2 * STOCH, BL], F32, tag="bod")
                nc.sync.dma_start(bod[:], d_bod.ap())

            # ---------- phase 0: xop = embed @ We (+bias), column-permuted ----------
            with tc.tile_pool(name="p0w", bufs=1) as p0w, \
                 tc.tile_pool(name="p0e", bufs=13) as p0e, \
                 tc.tile_pool(name="p0p", bufs=2, space="PSUM") as p0p:
                bias_o_sb = None
                if not affine_trivial:
                    bias_o_sb = p0w.tile([128, HIDDEN], F32, tag="biaso")
                    nc.sync.dma_start(bias_o_sb[:], d_bias_o.ap())
                wek = []
                for k in range(KT_E):
                    wt = p0w.tile([128, HIDDEN], BF16, tag=f"p0we{k}", name=f"p0we{k}")
                    nc.sync.dma_start(wt[:], d_We.ap()[k * 128:(k + 1) * 128, :])
                    wek.append(wt)
                for mt in range(T * BL // 128):
                    ek = []
                    for k in range(KT_E):
                        ekt = p0e.tile([128, 128], BF16, tag="p0ek")
                        nc.sync.dma_start(ekt[:], d_embT.ap()[k * 128:(k + 1) * 128,
                                                              mt * 128:(mt + 1) * 128])
                        ek.append(ekt)
                    xop0 = spool.tile([128, HIDDEN], F32, tag="p0out")
                    for nt in range(2):
                        ps = p0p.tile([128, 512], F32, tag="p0ps")
                        for k in range(KT_E):
                            nc.tensor.matmul(
                                ps[:], ek[k][:],
                                wek[k][:, nt * 512:(nt + 1) * 512],
                                start=(k == 0), stop=(k == KT_E - 1))
                        # chunk c = 4*nt + j  ->  permuted col j*256 + nt*128
                        for j in range(4):
                            dst = xop0[:, j * 256 + nt * 128:j * 256 + nt * 128 + 128]
                            src = ps[:, j * 128:(j + 1) * 128]
                            if bias_o_sb is not None:
                                nc.vector.tensor_add(
                                    dst, src,
                                    bias_o_sb[:, j * 256 + nt * 128:j * 256 + nt * 128 + 128])
                            elif j % 2 == 0:
                                nc.scalar.copy(dst, src)
                            else:
                                nc.vector.tensor_copy(dst, src)
                    nc.sync.dma_start(s_xop.ap()[mt * 128:(mt + 1) * 128, :], xop0[:])

                # zero the pad regions of deterT_all
                zpad = spool.tile([128, 128], BF16, tag="zpad")
                nc.vector.memset(zpad[:], 0.0)
                for (_k, r0, r1, ts) in heads:
                    pr0 = r0 + len(ts) * BL
                    if pr0 < r1:
                        for kk in range(8):
                            nc.sync.dma_start(s_dTa.ap()[kk, :, pr0:r1], zpad[:, 0:r1 - pr0])

            # ---------- scan ----------
            with tc.tile_pool(name="gpool", bufs=2, space="PSUM") as gpool, \
                 tc.tile_pool(name="xpool", bufs=1, space="PSUM") as xpool, \
                 tc.tile_pool(name="opool", bufs=1, space="PSUM") as opool, \
                 tc.tile_pool(name="dpool", bufs=1, space="PSUM") as dpool, \
                 tc.tile_pool(name="tpool", bufs=2, space="PSUM") as tpool, \
                 tc.tile_pool(name="p2p", bufs=1, space="PSUM") as p2p, \
                 tc.tile_pool(name="p2w", bufs=2) as p2w, \
                 tc.tile_pool(name="p2d", bufs=10) as p2d:

                def rsqrt_dve(st, ps_tile, pcol, outcol, qsb, magic_w):
                    """st[:, outcol] = 1/sqrt(q - m^2 + eps); copy-based."""
                    nc.vector.tensor_copy(st[:, 4:6], ps_tile[:, pcol:pcol + 2])
                    m, q = st[:, 4:5], st[:, 5:6]
                    v = st[:, 0:1]
                    nc.vector.tensor_tensor(st[:, 1:2], m, m, ALU.mult)
                    nc.vector.scalar_tensor_tensor(v, q, 1e-5, st[:, 1:2],
                                                   ALU.add, ALU.subtract)
                    yi = st[:, outcol:outcol + 1]
                    nc.vector.tensor_scalar(st[:, 2:3].bitcast(I32), v.bitcast(I32),
                                            1, None, ALU.arith_shift_right)
                    nc.vector.scalar_tensor_tensor(yi.bitcast(I32), magic[:], 1,
                                                   st[:, 2:3].bitcast(I32),
                                                   ALU.mult, ALU.subtract)
                    h = st[:, 1:2]
                    for _ in range(NEWTON_ITERS):
                        nc.vector.tensor_tensor(h, yi, yi, ALU.mult)
                        nc.vector.tensor_tensor(h, h, v, ALU.mult)
                        nc.vector.tensor_scalar(h, h, -0.5, 1.5, ALU.mult, ALU.add)
                        nc.vector.tensor_tensor(yi, yi, h, ALU.mult)
                    return m, yi

                def rsqrt_sb(st, m, q, outcol, eps_done=False):
                    """st[:, outcol] = 1/sqrt(q - m^2 + eps); all-sbuf variant."""
                    v = st[:, 0:1]
                    nc.vector.tensor_tensor(st[:, 1:2], m, m, ALU.mult)
                    nc.vector.scalar_tensor_tensor(v, q, 1e-5, st[:, 1:2],
                                                   ALU.add, ALU.subtract)
                    yi = st[:, outcol:outcol + 1]
                    nc.vector.tensor_scalar(st[:, 2:3].bitcast(I32), v.bitcast(I32),
                                            1, None, ALU.arith_shift_right)
                    nc.vector.scalar_tensor_tensor(yi.bitcast(I32), magic[:], 1,
                                                   st[:, 2:3].bitcast(I32),
                                                   ALU.mult, ALU.subtract)
                    h = st[:, 1:2]
                    for _ in range(2):
                        nc.vector.tensor_tensor(h, yi, yi, ALU.mult)
                        nc.vector.tensor_tensor(h, h, v, ALU.mult)
                        nc.vector.tensor_scalar(h, h, -0.5, 1.5, ALU.mult, ALU.add)
                        nc.vector.tensor_tensor(yi, yi, h, ALU.mult)
                    return yi

                def emit_phase2_group(hk, mt):
                    """Ensemble prior head hk for row-group mt (filler work —
                    interleaved into scan idle). Raw pstd out (host softplus)."""
                    Wio = p2w.tile([128, 8 * HIDDEN], BF16, tag="Wio",
                                   name=f"Wio_{hk}_{mt}")
                    for kk in range(8):
                        nc.sync.dma_start(Wio[:, kk * 1024:(kk + 1) * 1024],
                                          d_Wio.ap()[hk, kk * 128:(kk + 1) * 128, :])
                    Wids = p2w.tile([128, 8 * 2 * STOCH], BF16, tag="Wids",
                                    name=f"Wids_{hk}_{mt}")
                    for kk in range(8):
                        nc.sync.dma_start(Wids[:, kk * 64:(kk + 1) * 64],
                                          d_Wids.ap()[hk, kk * 128:(kk + 1) * 128, :])
                    if not affine_trivial:
                        gbn_e = p2w.tile([128, 2 * HIDDEN], F32, tag="gbnE",
                                         name=f"gbnE_{hk}_{mt}")
                        nc.sync.dma_start(gbn_e[:], d_gbn_e.ap()[hk])
                        bias_e = p2w.tile([128, HIDDEN], F32, tag="biasE",
                                          name=f"biasE_{hk}_{mt}")
                        nc.sync.dma_start(bias_e[:], d_bias_e.ap()[hk])
                        bids = p2w.tile([128, 2 * STOCH], F32, tag="bidsE",
                                        name=f"bidsE_{hk}_{mt}")
                        nc.sync.dma_start(bids[:], d_bids.ap()[hk])
                    dk = []
                    for kk in range(8):
                        dkt = p2d.tile([128, 128], BF16, tag="p2dk",
                                       name=f"p2dk_{hk}_{mt}_{kk}")
                        nc.sync.dma_start(dkt[:],
                                          s_dTa.ap()[kk, :, mt * 128:(mt + 1) * 128])
                        dk.append(dkt)
                    z_e = spool.tile([128, HIDDEN], F32, tag="z_e")
                    estats = spool.tile([128, 8], F32, tag="estats")
                    for nt in range(2):
                        ps = p2p.tile([128, 512], F32, tag="p2ps")
                        for kk in range(8):
                            nc.tensor.matmul(
                                ps[:], dk[kk][:],
                                Wio[:, kk * 1024 + nt * 512: kk * 1024 + nt * 512 + 512],
                                start=(kk == 0), stop=(kk == 7))
                        if not affine_trivial:
                            nc.vector.tensor_add(z_e[:, nt * 512:(nt + 1) * 512], ps[:],
                                                 bias_e[:, nt * 512:(nt + 1) * 512])
                        else:
                            nc.scalar.copy(z_e[:, nt * 512:(nt + 1) * 512], ps[:])
                        nc.vector.reduce_sum(estats[:, 4 + nt:5 + nt],
                                             z_e[:, nt * 512:(nt + 1) * 512], axis=AX)
                        sqe = spool.tile([128, 512], F32, tag="sqt2")
                        nc.scalar.activation(sqe[:], z_e[:, nt * 512:(nt + 1) * 512],
                                             AF.Square,
                                             accum_out=estats[:, 6 + nt:7 + nt])
                    se = spool.tile([128, 8], F32, tag="lnst")
                    nc.vector.tensor_add(se[:, 4:5], estats[:, 4:5], estats[:, 5:6])
                    nc.vector.tensor_add(se[:, 5:6], estats[:, 6:7], estats[:, 7:8])
                    nc.vector.tensor_scalar_mul(se[:, 4:5], se[:, 4:5], 1.0 / HIDDEN)
                    nc.vector.tensor_scalar_mul(se[:, 5:6], se[:, 5:6], 1.0 / HIDDEN)
                    rstd_e = rsqrt_sb(se, se[:, 4:5], se[:, 5:6], 6)
                    nmr_e = se[:, 7:8]
                    nc.vector.scalar_tensor_tensor(nmr_e, se[:, 4:5], -1.0, rstd_e,
                                                   ALU.mult, ALU.mult)
                    hsb = spool.tile([128, HIDDEN], BF16, tag="hsb")
                    if affine_trivial:
                        ee = spool.tile([128, HIDDEN], F32, tag="ee")
                        nc.scalar.activation(ee[:], z_e[:], AF.Exp,
                                             bias=nmr_e, scale=rstd_e)
                        xre = spool.tile([128, HIDDEN], F32, tag="xre")
                        nc.vector.tensor_scalar(xre[:], z_e[:], rstd_e, nmr_e,
                                                ALU.mult, ALU.add)
                        nc.vector.tensor_scalar_max(xre[:], xre[:], 0.0)
                        nc.vector.scalar_tensor_tensor(hsb[:], ee[:], -1.0, xre[:],
                                                       ALU.add, ALU.min)
                    else:
                        eln = z_e
                        nc.vector.tensor_scalar(eln[:], z_e[:], rstd_e, nmr_e,
                                                ALU.mult, ALU.add)
                        nc.vector.tensor_mul(eln[:], eln[:], gbn_e[:, 0:HIDDEN])
                        nc.vector.tensor_add(eln[:], eln[:], gbn_e[:, HIDDEN:])
                        ee = spool.tile([128, HIDDEN], F32, tag="ee")
                        nc.scalar.activation(ee[:], eln[:], AF.Exp)
                        xre = spool.tile([128, HIDDEN], F32, tag="xre")
                        nc.vector.tensor_scalar_max(xre[:], eln[:], 0.0)
                        nc.vector.scalar_tensor_tensor(hsb[:], ee[:], -1.0, xre[:],
                                                       ALU.add, ALU.min)
                    pse_t = dpool.tile([128, 96], F32, tag="pd", name="pse_t")
                    pse = pse_t[:, 32:96]
                    for kk in range(8):
                        tpe = tpool.tile([128, 128], BF16, tag="tp",
                                         name=f"tpe_{hk}_{mt}_{kk}")
                        nc.tensor.transpose(tpe[:], hsb[:, kk * 128:(kk + 1) * 128],
                                            eye128b[:])
                        hst = spool.tile([128, 128], BF16, tag="hst")
                        nc.scalar.copy(hst[:], tpe[:])
                        nc.tensor.matmul(pse, hst[:],
                                         Wids[:, kk * 64:(kk + 1) * 64],
                                         start=(kk == 0), stop=(kk == 7))
                    ed = spool.tile([128, 2 * STOCH], F32, tag="ed")
                    nc.vector.tensor_copy(ed[:], pse)
                    if not affine_trivial:
                        nc.vector.tensor_add(ed[:], ed[:], bids[:])
                    for t2 in gts_by_group[(hk, mt)]:
                        lr = row_ofs[t2] - mt * 128
                        nc.sync.dma_start(o_ens.ap()[:, t2, :], ed[lr:lr + BL, :])

                deter = spool2.tile([128, 256], F32, tag="deterN")
                nc.vector.memset(deter[:], 0.0)
                dTm = [spool2.tile([128, 128], BF16, tag=f"dTm{i}",
                                   name=f"dTm_init{i}") for i in range(2)]
                nc.vector.memset(dTm[0][:], 0.0)
                nc.vector.memset(dTm[1][:], 0.0)
                in45T = spool2.tile([45, BL], BF16, tag="in45T")
                nc.vector.memset(in45T[0:STOCH, :], 0.0)
                nc.sync.dma_start(in45T[STOCH:, :], d_am1T.ap()[:, 0:BL])
                xop_f = spool2.tile([128, 256], F32, tag="xop_f")
                for s in range(4):
                    nc.sync.dma_start(xop_f[32 * s:32 * s + 16, :],
                                      s_xop.ap()[0:BL, s * 256:(s + 1) * 256])

                for t in range(T):
                    # ---- img MLP quad: x = elu(ln(in45 @ Wimg)) ----
                    px = xpool.tile([128, 264], F32, tag="px")
                    if t == 0:
                        # one-time: zero stale PSUM (possibly Inf/NaN) so the
                        # never-written garbage strips can't poison accums /
                        # selrep contractions
                        nc.vector.memset(px[:], 0.0)
                    for s in range(4):
                        nc.tensor.matmul(px[32 * s:32 * s + 16, 0:257],
                                         in45T[:], Wimg[:, s * 257:(s + 1) * 257],
                                         start=True, stop=True,
                                         tile_position=(0, 32 * s))

                    # ---- GRU quads, d-part first (only needs deterT(t-1)) ----
                    pg = [gpool.tile([128, 388], F32, tag="pg", name=f"pg{i}")
                          for i in range(2)]  # noqa
                    if t < 2:  # cover both rotation buffers of the pool
                        for i in range(2):
                            nc.vector.memset(pg[i][:], 0.0)

                    def gru_emit(kk, blk, scol, first, last):
                        for i in range(2):
                            for s4 in range(4):
                                c = 4 * i + s4
                                nc.tensor.matmul(
                                    pg[i][32 * s4:32 * s4 + 16, 0:385],
                                    blk[:, scol:scol + 16],
                                    Wg[:, (kk * 8 + c) * 385:(kk * 8 + c + 1) * 385],
                                    start=first, stop=last,
                                    tile_position=(0, 32 * s4))

                    for k in range(8):  # d-part: Wg k-tiles 8..15
                        gru_emit(8 + k, dTm[k // 4], 32 * (k % 4), k == 0, False)

                    # ---- img postprocess (overlaps GRU-d on PE) ----
                    st_i = spool2.tile([128, 12], F32, tag="st_i")
                    nc.vector.tensor_copy(st_i[:, 8:9], px[:, 256:257])
                    sq_i = spool.tile([128, 256], F32, tag="sqscratch", name="sq_i")
                    nc.scalar.activation(sq_i[:], px[:, 0:256], AF.Square,
                                         accum_out=st_i[:, 9:10])
                    nc.tensor.matmul(px[:, 260:262], selh[:], st_i[:, 8:10],
                                     start=True, stop=True)
                    m_i, rstd_i = rsqrt_dve(st_i, px, 260, 6,
                                            st_i[:, 9:10], magic_h[:])
                    nmr_i = st_i[:, 7:8]
                    nc.vector.scalar_tensor_tensor(nmr_i, m_i, -1.0, rstd_i,
                                                   ALU.mult, ALU.mult)
                    xb = spool.tile([128, 256], BF16, tag="xb")
                    if affine_trivial:
                        # exp(apply) on scalar || relu(apply) on vector
                        ex = spool.tile([128, 256], F32, tag="ex")
                        nc.scalar.activation(ex[:], px[:, 0:256], AF.Exp,
                                             bias=nmr_i, scale=rstd_i)
                        xr = spool.tile([128, 256], F32, tag="xr")
                        nc.vector.tensor_scalar(xr[:], px[:, 0:256], rstd_i,
                                                nmr_i, ALU.mult, ALU.add)
                        nc.vector.tensor_scalar_max(xr[:], xr[:], 0.0)
                        nc.vector.scalar_tensor_tensor(xb[:], ex[:], -1.0, xr[:],
                                                       ALU.add, ALU.min)
                    else:
                        xh = spool.tile([128, 256], F32, tag="xh")
                        nc.scalar.activation(xh[:], px[:, 0:256], AF.Identity,
                                             bias=nmr_i, scale=rstd_i)
                        nc.vector.tensor_mul(xh[:], xh[:], gbn_i[:, 0:256])
                        nc.vector.tensor_add(xh[:], xh[:], gbn_i[:, 256:512])
                        ex = spool.tile([128, 256], F32, tag="ex")
                        nc.scalar.activation(ex[:], xh[:], AF.Exp)
                        xr = spool.tile([128, 256], F32, tag="xr")
                        nc.vector.tensor_scalar_max(xr[:], xh[:], 0.0)
                        nc.vector.scalar_tensor_tensor(xb[:], ex[:], -1.0, xr[:],
                                                       ALU.add, ALU.min)
                    xT = []
                    for i in range(2):
                        tp = tpool.tile([128, 128], BF16, tag="tp", name=f"tpx{i}")
                        nc.tensor.transpose(tp[:], xb[:, i * 128:(i + 1) * 128],
                                            eye128b[:])
                        xt = spool.tile([128, 128], BF16, tag=f"xT{i}")
                        nc.vector.tensor_copy(xt[:], tp[:])
                        xT.append(xt)

                    # ---- GRU x-part quads ----
                    for k in range(8):
                        gru_emit(k, xT[k // 4], 32 * (k % 4), False, k == 7)

                    # phase-2 filler emitted here: behind this step's critical
                    # img chain in queue order, ahead of the gates idle window
                    for (hk_, mt_) in group_ready.get(t - 1, []):
                        emit_phase2_group(hk_, mt_)

                    # ---- GRU stats ----
                    st_g = spool2.tile([128, 12], F32, tag="st_g")
                    nc.vector.tensor_copy(st_g[:, 8:9], pg[0][:, 384:385])
                    sq_g = spool.tile([128, 384], F32, tag="sqg")
                    nc.scalar.activation(sq_g[:], pg[0][:, 0:384], AF.Square,
                                         accum_out=st_g[:, 10:11])
                    zg1 = spool.tile([128, 384], F32, tag="zg1")
                    nc.vector.tensor_copy(zg1[:], pg[1][:, 0:384])
                    sq_g2 = spool.tile([128, 384], F32, tag="sqg2")
                    nc.vector.scalar_tensor_tensor(sq_g2[:], zg1[:], 1.0, zg1[:],
                                                   ALU.mult, ALU.mult,
                                                   accum_out=st_g[:, 11:12])
                    st_in = spool2.tile([128, 2], F32, tag="st_in")
                    nc.vector.tensor_add(st_in[:, 0:1], st_g[:, 8:9],
                                         pg[1][:, 384:385])
                    nc.vector.tensor_add(st_in[:, 1:2], st_g[:, 10:11], st_g[:, 11:12])
                    nc.tensor.matmul(pg[0][:, 385:387], selg[:], st_in[:],
                                     start=True, stop=True)
                    m_g, rstd_g = rsqrt_dve(st_g, pg[0], 385, 6,
                                            st_in[:, 1:2], magic_g[:])
                    mr_g = st_g[:, 7:8]
                    nc.vector.tensor_tensor(mr_g, m_g, rstd_g, ALU.mult)
                    gs = spool2.tile([128, 4], F32, tag="gs")
                    nc.vector.tensor_scalar_mul(gs[:, 0:1], rstd_g, 0.5)
                    nc.vector.tensor_scalar_mul(gs[:, 1:2], mr_g, -0.5)
                    nc.vector.tensor_scalar(gs[:, 2:3], mr_g, -0.5,
                                            0.5 * UPDATE_BIAS, ALU.mult, ALU.add)

                    if not affine_trivial:
                        zn = [spool.tile([128, 384], F32, tag=f"zn{i}",
                                         name=f"zn_{i}") for i in range(2)]
                        for i in range(2):
                            nc.vector.tensor_scalar(zn[i][:], pg[i][:, 0:384],
                                                    rstd_g, mr_g,
                                                    ALU.mult, ALU.subtract)
                            nc.vector.tensor_mul(zn[i][:], zn[i][:],
                                                 gbn_g[:, (2 * i) * 384:(2 * i + 1) * 384])
                            nc.vector.tensor_add(zn[i][:], zn[i][:],
                                                 gbn_g[:, (2 * i + 1) * 384:(2 * i + 2) * 384])
                        gate_src = [zn[0][:], zn[1][:]]
                    else:
                        gate_src = [pg[0][:], pg[1][:]]

                    # ---- gates (sigmoid via tanh) ----
                    th_r = spool.tile([128, 256], F32, tag="th_r")
                    cn = spool.tile([128, 256], F32, tag="cn")
                    cand = spool.tile([128, 256], F32, tag="cand")
                    th_u = spool.tile([128, 256], F32, tag="th_u")
                    for i in range(2):
                        sl = slice(i * 128, (i + 1) * 128)
                        src = gate_src[i]
                        if affine_trivial:
                            nc.scalar.activation(th_r[:, sl], src[:, 0:128], AF.Tanh,
                                                 bias=gs[:, 1:2], scale=gs[:, 0:1])
                            nc.vector.tensor_scalar(cn[:, sl], src[:, 128:256],
                                                    rstd_g, mr_g,
                                                    ALU.mult, ALU.subtract)
                            nc.scalar.activation(th_u[:, sl], src[:, 256:384], AF.Tanh,
                                                 bias=gs[:, 2:3], scale=gs[:, 0:1])
                        else:
                            nc.scalar.activation(th_r[:, sl], src[:, 0:128], AF.Tanh,
                                                 scale=0.5)
                            nc.vector.tensor_copy(cn[:, sl], src[:, 128:256])
                            nc.scalar.activation(th_u[:, sl], src[:, 256:384], AF.Tanh,
                                                 bias=0.5 * UPDATE_BIAS, scale=0.5)
                    nc.vector.scalar_tensor_tensor(cn[:], th_r[:], 1.0,
                                                   cn[:], ALU.add, ALU.mult)
                    nc.vector.tensor_scalar(th_u[:], th_u[:], 0.5, 0.5,
                                            ALU.mult, ALU.add)
                    nc.scalar.activation(cand[:], cn[:], AF.Tanh, scale=0.5)

                    prev = deter
                    deter = spool2.tile([128, 256], F32, tag="deterN")
                    m_t = maskF[:, t:t + 1]
                    t1 = cn  # reuse
                    nc.vector.scalar_tensor_tensor(t1[:], prev[:], m_t, cand[:],
                                                   ALU.mult, ALU.subtract)
                    nc.vector.tensor_mul(t1[:], t1[:], th_u[:])
                    nc.vector.scalar_tensor_tensor(deter[:], prev[:], m_t, t1[:],
                                                   ALU.mult, ALU.subtract)
                    nc.sync.dma_start(o_detF.ap()[:, t, :], deter[:])

                    # deterT blocks: unmasked (obs, phase2) + masked (next GRU-d)
                    db = spool.tile([128, 256], BF16, tag="db")
                    nc.vector.tensor_copy(db[:], deter[:])
                    dbm = spool.tile([128, 256], BF16, tag="dbm")
                    nc.scalar.activation(dbm[:], deter[:], AF.Copy,
                                         scale=maskF[:, t + 1:t + 2])
                    dTu = []
                    dTm = []
                    for i in range(2):
                        tpu = tpool.tile([128, 128], BF16, tag="tp", name=f"tpdu{i}")
                        nc.tensor.transpose(tpu[:], db[:, i * 128:(i + 1) * 128],
                                            eye128b[:])
                        du = spool2.tile([128, 128], BF16, tag=f"dTu{i}")
                        if i == 0:
                            nc.vector.tensor_copy(du[:], tpu[:])
                        else:
                            nc.scalar.copy(du[:], tpu[:])
                        dTu.append(du)
                        tpm = tpool.tile([128, 128], BF16, tag="tp", name=f"tpdm{i}")
                        nc.tensor.transpose(tpm[:], dbm[:, i * 128:(i + 1) * 128],
                                            eye128b[:])
                        dm_ = spool2.tile([128, 128], BF16, tag=f"dTm{i}")
                        nc.scalar.copy(dm_[:], tpm[:])
                        dTm.append(dm_)
                    # feed phase 2 (unmasked)
                    for c in range(8):
                        eng = nc.gpsimd if c % 2 == 0 else nc.sync
                        eng.dma_start(s_dTa.ap()[c, :, row_ofs[t]:row_ofs[t] + BL],
                                      dTu[c // 4][:, 32 * (c % 4):32 * (c % 4) + 16])

                    # ---- obs posterior quads ----
                    po = opool.tile([128, 260], F32, tag="po")
                    if t == 0:
                        nc.vector.memset(po[:], 0.0)
                    for k in range(8):
                        blk, s = dTu[k // 4], k % 4
                        for s4 in range(4):
                            nc.tensor.matmul(
                                po[32 * s4:32 * s4 + 16, 0:256],
                                blk[:, 32 * s:32 * s + 16],
                                Wd[:, (k * 4 + s4) * 256:(k * 4 + s4 + 1) * 256],
                                start=(k == 0), stop=(k == 7),
                                tile_position=(0, 32 * s4))
                    z_o = spool.tile([128, 256], F32, tag="z_o")
                    st_o = spool2.tile([128, 12], F32, tag="st_o")
                    nc.vector.scalar_tensor_tensor(z_o[:], po[:, 0:256], 1.0,
                                                   xop_f[:], ALU.mult, ALU.add,
                                                   accum_out=st_o[:, 8:9])
                    sq_o = spool.tile([128, 256], F32, tag="sqscratch", name="sq_o")
                    nc.vector.scalar_tensor_tensor(sq_o[:], z_o[:], 1.0, z_o[:],
                                                   ALU.mult, ALU.mult,
                                                   accum_out=st_o[:, 9:10])
                    nc.tensor.matmul(po[:, 256:258], selh[:], st_o[:, 8:10],
                                     start=True, stop=True)
                    m_o, rstd_o = rsqrt_dve(st_o, po, 256, 6,
                                            st_o[:, 9:10], magic_h[:])
                    nmr_o = st_o[:, 7:8]
                    nc.vector.scalar_tensor_tensor(nmr_o, m_o, -1.0, rstd_o,
                                                   ALU.mult, ALU.mult)
                    xob = spool.tile([128, 256], BF16, tag="xob")
                    if affine_trivial:
                        eo = spool.tile([128, 256], F32, tag="ex", name="eo")
                        nc.scalar.activation(eo[:], z_o[:], AF.Exp,
                                             bias=nmr_o, scale=rstd_o)
                        xro = spool.tile([128, 256], F32, tag="xr", name="xro")
                        nc.vector.tensor_scalar(xro[:], z_o[:], rstd_o,
                                                nmr_o, ALU.mult, ALU.add)
                        nc.vector.tensor_scalar_max(xro[:], xro[:], 0.0)
                        nc.vector.scalar_tensor_tensor(xob[:], eo[:], -1.0, xro[:],
                                                       ALU.add, ALU.min)
                    else:
                        oh = z_o
                        nc.scalar.activation(oh[:], z_o[:], AF.Identity,
                                             bias=nmr_o, scale=rstd_o)
                        nc.vector.tensor_mul(oh[:], oh[:], gbn_o[:, 0:256])
                        nc.vector.tensor_add(oh[:], oh[:], gbn_o[:, 256:512])
                        eo = spool.tile([128, 256], F32, tag="ex", name="eo")
                        nc.scalar.activation(eo[:], oh[:], AF.Exp)
                        xro = spool.tile([128, 256], F32, tag="xr", name="xro")
                        nc.vector.tensor_scalar_max(xro[:], oh[:], 0.0)
                        nc.vector.scalar_tensor_tensor(xob[:], eo[:], -1.0, xro[:],
                                                       ALU.add, ALU.min)
                    xoT = []
                    for i in range(2):
                        tp = tpool.tile([128, 128], BF16, tag="tp", name=f"tpxo{i}")
                        nc.tensor.transpose(tp[:], xob[:, i * 128:(i + 1) * 128],
                                            eye128b[:])
                        xo_t = spool.tile([128, 128], BF16, tag=f"xoT{i}")
                        nc.vector.tensor_copy(xo_t[:], tp[:])
                        xoT.append(xo_t)

                    # ---- obs dist (feature-major): distT[2S, BL] ----
                    pd = dpool.tile([128, 96], F32, tag="pd")
                    for c in range(8):
                        pos = 0 if c < 4 else 64
                        nc.tensor.matmul(
                            pd[pos:pos + 64, 0:16],
                            Wod[:, c * 64:(c + 1) * 64],
                            xoT[c // 4][:, 32 * (c % 4):32 * (c % 4) + 16],
                            start=(c % 4 == 0), stop=(c % 4 == 3),
                            tile_position=(0, pos))
                    dc0 = spool.tile([64, BL], F32, tag="dc0")
                    dc1 = spool.tile([64, BL], F32, tag="dc1")
                    nc.vector.tensor_copy(dc0[:], pd[0:64, 0:16])
                    nc.scalar.copy(dc1[:], pd[64:128, 0:16])
                    distT = spool.tile([64, BL], F32, tag="distT")
                    nc.vector.tensor_add(distT[:], dc0[:], dc1[:])
                    if not affine_trivial:
                        nc.vector.tensor_add(distT[:], distT[:], bod[:])
                    nc.sync.dma_start(o_meanT.ap()[:, t, :], distT[0:STOCH, :])
                    nc.sync.dma_start(o_rawsT.ap()[:, t, :], distT[STOCH:, :])

                    # ---- next-step in45T + xop prefetch ----
                    if t + 1 < T:
                        in45T = spool2.tile([45, BL], BF16, tag="in45T")
                        nc.vector.tensor_tensor(
                            in45T[0:STOCH, :], distT[0:STOCH, :],
                            maskTb[:, (t + 1) * BL:(t + 2) * BL], ALU.mult)
                        nc.gpsimd.dma_start(in45T[STOCH:, :],
                                            d_am1T.ap()[:, (t + 1) * BL:(t + 2) * BL])
                        xop_f = spool2.tile([128, 256], F32, tag="xop_f")
                        for s in range(4):
                            nc.gpsimd.dma_start(
                                xop_f[32 * s:32 * s + 16, :],
                                s_xop.ap()[(t + 1) * BL:(t + 2) * BL,
                                           s * 256:(s + 1) * 256])
                    if t == T - 1:
                        for (hk_, mt_) in group_ready.get(t, []):
                            emit_phase2_group(hk_, mt_)


    nc.compile()
    return nc


def _fold_rows(a):
    """[16, X] -> [128, X]: batch rows replicated at partitions 32s+b."""
    out = np.zeros((128,) + a.shape[1:], a.dtype)
    for s in range(4):
        out[32 * s:32 * s + 16] = a
    return out


def kernel(embed, action, is_first, ens_index,
           W_img_in, b_img_in, g_img_in, bn_img_in,
           W_gru, b_gru, g_gru, bn_gru,
           W_img_out, b_img_out, g_img_out, bn_img_out,
           W_img_dist, b_img_dist,
           W_obs_out, b_obs_out, g_obs_out, bn_obs_out,
           W_obs_dist, b_obs_dist):
    embed = np.asarray(embed, np.float32)
    action = np.asarray(action, np.float32)
    is_first = np.asarray(is_first)
    ens_index = np.asarray(ens_index, np.int64)

    affine_trivial = bool(
        np.all(np.asarray(b_img_in) == 0) and np.all(np.asarray(g_img_in) == 1)
        and np.all(np.asarray(bn_img_in) == 0)
        and np.all(np.asarray(b_gru) == 0) and np.all(np.asarray(g_gru) == 1)
        and np.all(np.asarray(bn_gru) == 0)
        and np.all(np.asarray(b_img_out) == 0) and np.all(np.asarray(g_img_out) == 1)
        and np.all(np.asarray(bn_img_out) == 0)
        and np.all(np.asarray(b_img_dist) == 0)
        and np.all(np.asarray(b_obs_out) == 0) and np.all(np.asarray(g_obs_out) == 1)
        and np.all(np.asarray(bn_obs_out) == 0)
        and np.all(np.asarray(b_obs_dist) == 0))

    key = (tuple(int(x) for x in ens_index), affine_trivial)
    if key not in _CACHE:
        _CACHE[key] = _build(ens_index, affine_trivial)
    nc = _CACHE[key]

    bf = lambda a: np.ascontiguousarray(np.asarray(a, np.float32)).astype(ml_dtypes.bfloat16)
    f32 = np.float32
    mask = 1.0 - np.asarray(is_first, np.float32)          # [B, T]
    a_m = action * mask[:, :, None]                        # [B, T, ACT]

    # ---- weight rearrangement (shared across cores) ----
    Wg_full = np.asarray(W_gru, f32)                       # [2048, 3072]
    v = Wg_full.reshape(16, 128, 3, 8, 128)                # k, kp, g, c, lf
    Wg_r = np.zeros((128, 16 * 8 * 385), f32)
    for k in range(16):
        for c in range(8):
            blk = v[k, :, :, c, :].reshape(128, 384)       # kp, (g,lf)
            col = (k * 8 + c) * 385
            Wg_r[:, col:col + 384] = blk
            Wg_r[:, col + 384] = blk.sum(1)
    Wd_full = np.asarray(W_obs_out, f32)[:DETER]           # [1024, 1024]
    vd = Wd_full.reshape(8, 128, 8, 128)                   # k, kp, c, lf
    Wd_r = np.zeros((128, 8 * 4 * 256), f32)
    for k in range(8):
        for s in range(4):
            col = (k * 4 + s) * 256
            Wd_r[:, col:col + 128] = vd[k, :, s, :]
            Wd_r[:, col + 128:col + 256] = vd[k, :, s + 4, :]
    Wimg_full = np.concatenate([np.asarray(W_img_in, f32),
                                np.asarray(b_img_in, f32)[None, :]], 0)  # [45, 1024]
    vi = Wimg_full.reshape(45, 8, 128)
    Wimg_r = np.zeros((45, 4 * 257), f32)
    for s in range(4):
        col = s * 257
        blk = np.concatenate([vi[:, s, :], vi[:, s + 4, :]], 1)          # [45, 256]
        Wimg_r[:, col:col + 256] = blk
        Wimg_r[:, col + 256] = blk.sum(1)
    Wod_full = np.asarray(W_obs_dist, f32)                 # [1024, 64]
    Wod_r = np.concatenate([Wod_full[c * 128:(c + 1) * 128, :] for c in range(8)], 1)

    selg = np.zeros((128, 128), f32)
    selh = np.zeros((128, 128), f32)
    for p in range(16):
        for s in range(4):
            for s2 in range(4):
                selg[32 * s + p, 32 * s2 + p] = 1.0 / (3 * DETER)
                selh[32 * s + p, 32 * s2 + p] = 1.0 / HIDDEN

    def fold_cols(vec1024):
        """[1024] -> [128, 256] folded (chunk c at strip c%4, colgrp c//4)."""
        vv = np.asarray(vec1024, f32).reshape(8, 128)
        out = np.zeros((128, 256), f32)
        for c in range(8):
            out[32 * (c % 4):32 * (c % 4) + 16,
                (c // 4) * 128:(c // 4) * 128 + 128] = vv[c]
        return out

    def fold_gru(vec3072):
        """[3072] -> [2, 128, 384] per-bank folded (g,lf cols; chunk 4i+s)."""
        vv = np.asarray(vec3072, f32).reshape(3, 8, 128)   # g, c, lf
        out = np.zeros((2, 128, 384), f32)
        for c in range(8):
            i, s = c // 4, c % 4
            out[i, 32 * s:32 * s + 16, :] = vv[:, c, :].reshape(384)
        return out

    gg, gbn = fold_gru(g_gru), fold_gru(bn_gru)
    gbn_gru = np.zeros((128, 4 * 384), f32)
    for i in range(2):
        gbn_gru[:, (2 * i) * 384:(2 * i + 1) * 384] = gg[i]
        gbn_gru[:, (2 * i + 1) * 384:(2 * i + 2) * 384] = gbn[i]

    # bias_obs column-permuted to match s_xop layout (chunk c -> (c%4)*256+(c//4)*128)
    bo = np.asarray(b_obs_out, f32).reshape(8, 128)
    bias_obs_p = np.zeros((1, HIDDEN), f32)
    for c in range(8):
        bias_obs_p[0, (c % 4) * 256 + (c // 4) * 128:(c % 4) * 256 + (c // 4) * 128 + 128] = bo[c]
    bias_obs_p = np.tile(bias_obs_p, (128, 1))

    rep = {
        "We": bf(np.asarray(W_obs_out, f32)[DETER:]),
        "Wg": bf(Wg_r), "Wd": bf(Wd_r), "Wimg": bf(Wimg_r), "Wod": bf(Wod_r),
        "Wio": bf(W_img_out), "Wids": bf(W_img_dist),
        "selg": selg, "selh": selh,
        "eye128b": np.eye(128, dtype=f32).astype(ml_dtypes.bfloat16),
        "eye128": np.eye(128, dtype=f32),
        "gbn_gru": gbn_gru,
        "gbn_obs": np.concatenate([fold_cols(g_obs_out), fold_cols(bn_obs_out)], 1),
        "gbn_img": np.concatenate([fold_cols(g_img_in), fold_cols(bn_img_in)], 1),
        "gbn_ens": np.ascontiguousarray(np.tile(np.concatenate(
            [np.asarray(g_img_out, f32), np.asarray(bn_img_out, f32)], 1)[:, None, :],
            (1, 128, 1))),
        "bias_obs": bias_obs_p,
        "bias_ens": np.ascontiguousarray(
            np.tile(np.asarray(b_img_out, f32)[:, None, :], (1, 128, 1))),
        "b_od": np.ascontiguousarray(
            np.tile(np.asarray(b_obs_dist, f32)[:, None], (1, BL))),
        "b_ids": np.ascontiguousarray(
            np.tile(np.asarray(b_img_dist, f32)[:, None, :], (1, 128, 1))),
    }
    in_maps = []
    for j in range(NC):
        sl = slice(j * BL, (j + 1) * BL)
        e = embed[sl]                                      # [16, T, EMB]
        embT = np.ascontiguousarray(e.transpose(2, 1, 0).reshape(EMB, T * BL))
        am1 = np.concatenate([a_m[sl], np.ones((BL, T, 1), f32)], -1)
        am1T = np.ascontiguousarray(am1.transpose(2, 1, 0).reshape(ACT + 1, T * BL))
        m_j = np.concatenate([mask[sl], np.ones((BL, 1), f32)], 1)  # [16, T+1]
        maskF = _fold_rows(m_j)                            # [128, T+1]
        maskTb = np.tile(np.ascontiguousarray(m_j.T).reshape(-1)[None, :],
                         (STOCH, 1))                       # [32, (T+1)*16]
        im = dict(rep)
        im["embT"] = bf(embT)
        im["am1T"] = bf(am1T)
        im["maskF"] = np.ascontiguousarray(maskF)
        im["maskTb"] = np.ascontiguousarray(maskTb, dtype=f32)
        in_maps.append(im)

    global _LAST_IN_MAPS
    _LAST_IN_MAPS = in_maps
    res = bass_utils.run_bass_kernel_spmd(nc, in_maps, core_ids=list(range(NC)))

    # ---- host-side unfold/assembly ----
    def softplus(x):
        return np.logaddexp(0.0, x) + MIN_STD

    omean = np.empty((B, T, STOCH), f32)
    ostd = np.empty((B, T, STOCH), f32)
    deter_full = np.empty((B, T, DETER), f32)
    o_ens_full = np.concatenate([res.results[j]["o_ens"] for j in range(NC)], 0)
    for j in range(NC):
        r = res.results[j]
        sl = slice(j * BL, (j + 1) * BL)
        omean[sl] = r["o_meanT"].transpose(2, 1, 0)        # [32, T, 16] -> [16, T, 32]
        ostd[sl] = softplus(r["o_rawsT"].transpose(2, 1, 0))
        dF = r["o_detF"]                                   # [128, T, 256]
        arr = dF.reshape(4, 32, T, 2, 128)[:, :16]         # s, b, t, i, lf
        deter_full[sl] = arr.transpose(1, 2, 3, 0, 4).reshape(BL, T, DETER)

    pmean = o_ens_full[..., :STOCH]
    pstd = softplus(o_ens_full[..., STOCH:])
    out = np.concatenate([omean, ostd, omean, pmean, pstd, pmean, deter_full], -1)
    return np.ascontiguousarray(out, dtype=np.float32)


# revision 34
# speedup vs baseline: 1.0034x; 1.0034x over previous
"""EnsembleRSSM observe kernel for 8 Trainium2 NeuronCores.

Strategy: data-parallel over batch (B=128 -> 16 rows/core), zero cross-core
communication. The sequential T=64 scan runs per-core on its batch shard with
all scan weights resident in SBUF (bf16 matmul operands, fp32 PSUM/vector
math).

v2: "folded" layout — every [16, F] activation is stored [128, F/4] with
batch in the low 16 rows of each 32-partition strip and feature chunks
spread over 4 strips x column groups (chunk c -> strip c%4, colgroup c//4).
Matmuls use tile_position column tiling (4 concurrent streams -> ~4x PE
throughput at M=16), vector/scalar elementwise ops run at full 128-partition
occupancy, PE transposes shrink to [128,128] blocks (one per colgroup),
LayerNorm strip-reduction+broadcast is one constant "selrep" matmul, rsqrt is
a DVE bit-trick + Newton (no act-table switches: the whole scan runs on the
exp table with sigmoid written as tanh), and softplus is deferred to a
batched post-pass.  The obs-dist head is computed feature-major so the stoch
recurrence needs no extra transpose; its transposed output layout is fixed up
on the host.
"""
import sys

sys.path.insert(0, "/opt/trn_rl_repo")

import numpy as np
import ml_dtypes

import concourse.bass as bass
import concourse.bacc as bacc
import concourse.mybir as mybir
import concourse.tile as tile
from concourse import bass_utils

_orig_get_tables = bacc.get_activation_tables

def _filtered_tables(arch):
    tabs = _orig_get_tables(arch)
    keep = ("exp_and_others",)
    if not all(k in tabs for k in keep):
        return tabs
    return {k: (v if k in keep else set()) for k, v in tabs.items()}

bacc.get_activation_tables = _filtered_tables

B, T = 128, 64
EMB, ACT = 1536, 12
STOCH, DETER, HIDDEN, ENS = 32, 1024, 1024, 5
MIN_STD = 0.1
UPDATE_BIAS = -1.0
NC = 8
BL = B // NC  # 16 rows per core

F32 = mybir.dt.float32
BF16 = mybir.dt.bfloat16
I32 = mybir.dt.int32
AF = mybir.ActivationFunctionType
ALU = mybir.AluOpType
AX = mybir.AxisListType.X

MAGIC = 0x5F3759DF
# seed magics for rsqrt computed from W*q (SBUF raw sums) instead of q:
# rsqrt(q) with input W*q needs magic + log2(W)*2^22
MAGIC_G = MAGIC + int(round((np.log2(3 * 1024)) * (1 << 22)))
MAGIC_H = MAGIC + int(round((np.log2(1024)) * (1 << 22)))
NEWTON_ITERS = 1

_CACHE = {}
_LAST_IN_MAPS = None


def _row_layout(ens_index):
    """Head-sorted, 128-padded row layout for phase 2."""
    order = {}
    for t in range(T):
        order.setdefault(int(ens_index[t]), []).append(t)
    row_ofs = [0] * T
    heads = []
    r = 0
    for k in range(ENS):
        ts = order.get(k, [])
        r0 = r
        for t in ts:
            row_ofs[t] = r
            r += BL
        r = ((r + 127) // 128) * 128
        heads.append((k, r0, r, ts))
    return row_ofs, heads, max(r, 128)


def _build(ens_index, affine_trivial):
    nc = bacc.Bacc("TRN2", target_bir_lowering=False, debug=False,
                   num_devices=NC, detect_race_conditions=False)
    for cval in sorted({-1.0, -0.5, 0.5, 1.0, 1e-5, 0.5 * UPDATE_BIAS, MIN_STD}):
        th = nc.alloc_sbuf_tensor(f"constx-{cval}", [128, 1], F32)
        nc.gpsimd.memset(th.ap(), cval)
        nc.const_aps.aps[(F32, cval)] = th.ap()

    # ---- DRAM inputs ----
    d_embT = nc.dram_tensor("embT", [EMB, T * BL], BF16, kind="ExternalInput")
    d_We = nc.dram_tensor("We", [EMB, HIDDEN], BF16, kind="ExternalInput")
    d_Wg = nc.dram_tensor("Wg", [128, 16 * 8 * 385], BF16, kind="ExternalInput")
    d_Wd = nc.dram_tensor("Wd", [128, 8 * 4 * 256], BF16, kind="ExternalInput")
    d_Wimg = nc.dram_tensor("Wimg", [45, 4 * 257], BF16, kind="ExternalInput")
    d_Wod = nc.dram_tensor("Wod", [128, 8 * 64], BF16, kind="ExternalInput")
    d_Wio = nc.dram_tensor("Wio", [ENS, DETER, HIDDEN], BF16, kind="ExternalInput")
    d_Wids = nc.dram_tensor("Wids", [ENS, HIDDEN, 2 * STOCH], BF16, kind="ExternalInput")
    d_maskF = nc.dram_tensor("maskF", [128, T + 1], F32, kind="ExternalInput")
    d_maskTb = nc.dram_tensor("maskTb", [STOCH, (T + 1) * BL], F32, kind="ExternalInput")
    d_am1T = nc.dram_tensor("am1T", [ACT + 1, T * BL], BF16, kind="ExternalInput")
    d_selg = nc.dram_tensor("selg", [128, 128], F32, kind="ExternalInput")
    d_selh = nc.dram_tensor("selh", [128, 128], F32, kind="ExternalInput")
    d_eye128b = nc.dram_tensor("eye128b", [128, 128], BF16, kind="ExternalInput")
    d_eye128 = nc.dram_tensor("eye128", [128, 128], F32, kind="ExternalInput")
    # non-trivial affine support
    d_gbn_g = nc.dram_tensor("gbn_gru", [128, 4 * 384], F32, kind="ExternalInput")
    d_gbn_o = nc.dram_tensor("gbn_obs", [128, 2 * 256], F32, kind="ExternalInput")
    d_gbn_i = nc.dram_tensor("gbn_img", [128, 2 * 256], F32, kind="ExternalInput")
    d_bod = nc.dram_tensor("b_od", [2 * STOCH, BL], F32, kind="ExternalInput")
    d_gbn_e = nc.dram_tensor("gbn_ens", [ENS, 128, 2 * HIDDEN], F32, kind="ExternalInput")
    d_bias_o = nc.dram_tensor("bias_obs", [128, HIDDEN], F32, kind="ExternalInput")
    d_bias_e = nc.dram_tensor("bias_ens", [ENS, 128, HIDDEN], F32, kind="ExternalInput")
    d_bids = nc.dram_tensor("b_ids", [ENS, 128, 2 * STOCH], F32, kind="ExternalInput")

    o_detF = nc.dram_tensor("o_detF", [128, T, 256], F32, kind="ExternalOutput")
    o_meanT = nc.dram_tensor("o_meanT", [STOCH, T, BL], F32, kind="ExternalOutput")
    o_rawsT = nc.dram_tensor("o_rawsT", [STOCH, T, BL], F32, kind="ExternalOutput")
    o_ens = nc.dram_tensor("o_ens", [BL, T, 2 * STOCH], F32, kind="ExternalOutput")

    row_ofs, heads, R = _row_layout(ens_index)
    s_xop = nc.dram_tensor("xo_pre", [T * BL, HIDDEN], F32)     # column-permuted
    s_dTa = nc.dram_tensor("deterT_all", [8, 128, R], BF16)
    # phase-2 groups become ready when their last timestep's deterT lands
    group_ready = {}
    gts_by_group = {}
    for (hk, r0, r1, ts) in heads:
        for mt in range(r0 // 128, r1 // 128):
            i0 = (mt * 128 - r0) // BL
            gts = ts[i0:i0 + 8]
            if gts:
                group_ready.setdefault(max(gts), []).append((hk, mt))
                gts_by_group[(hk, mt)] = gts

    KT_E = EMB // 128  # 12

    with tile.TileContext(nc) as tc:
        with tc.tile_pool(name="wpool", bufs=1) as wpool, \
             tc.tile_pool(name="spool", bufs=1) as spool, \
             tc.tile_pool(name="spool2", bufs=2) as spool2:

            # ---------- resident weights ----------
            Wg = wpool.tile([128, 16 * 8 * 385], BF16, tag="Wg")
            for k in range(16):
                nc.sync.dma_start(Wg[:, k * 3080:(k + 1) * 3080],
                                  d_Wg.ap()[:, k * 3080:(k + 1) * 3080])
            Wd = wpool.tile([128, 8 * 4 * 256], BF16, tag="Wd")
            nc.sync.dma_start(Wd[:], d_Wd.ap())
            Wimg = wpool.tile([45, 4 * 257], BF16, tag="Wimg")
            nc.sync.dma_start(Wimg[:], d_Wimg.ap())
            Wod = wpool.tile([128, 8 * 64], BF16, tag="Wod")
            nc.sync.dma_start(Wod[:], d_Wod.ap())
            selg = wpool.tile([128, 128], F32, tag="selg")
            nc.sync.dma_start(selg[:], d_selg.ap())
            selh = wpool.tile([128, 128], F32, tag="selh")
            nc.sync.dma_start(selh[:], d_selh.ap())
            eye128b = wpool.tile([128, 128], BF16, tag="eye128b")
            nc.sync.dma_start(eye128b[:], d_eye128b.ap())
            eye128 = wpool.tile([128, 128], F32, tag="eye128")
            nc.sync.dma_start(eye128[:], d_eye128.ap())
            maskF = wpool.tile([128, T + 1], F32, tag="maskF")
            nc.sync.dma_start(maskF[:], d_maskF.ap())
            maskTb = wpool.tile([STOCH, (T + 1) * BL], F32, tag="maskTb")
            nc.sync.dma_start(maskTb[:], d_maskTb.ap())
            magic = wpool.tile([128, 1], I32, tag="magic")
            nc.vector.memset(magic[:], MAGIC)
            magic_g = wpool.tile([128, 1], I32, tag="magic_g")
            nc.vector.memset(magic_g[:], MAGIC_G)
            magic_h = wpool.tile([128, 1], I32, tag="magic_h")
            nc.vector.memset(magic_h[:], MAGIC_H)
            if not affine_trivial:
                gbn_g = wpool.tile([128, 4 * 384], F32, tag="gg")
                nc.sync.dma_start(gbn_g[:], d_gbn_g.ap())
                gbn_o = wpool.tile([128, 2 * 256], F32, tag="go")
                nc.sync.dma_start(gbn_o[:], d_gbn_o.ap())
                gbn_i = wpool.tile([128, 2 * 256], F32, tag="gi")
                nc.sync.dma_start(gbn_i[:], d_gbn_i.ap())
                bod = wpool.tile([2 * STOCH, BL], F32, tag="bod")
                nc.sync.dma_start(bod[:], d_bod.ap())

            # ---------- phase 0: xop = embed @ We (+bias), column-permuted ----------
            with tc.tile_pool(name="p0w", bufs=1) as p0w, \
                 tc.tile_pool(name="p0e", bufs=13) as p0e, \
                 tc.tile_pool(name="p0p", bufs=2, space="PSUM") as p0p:
                bias_o_sb = None
                if not affine_trivial:
                    bias_o_sb = p0w.tile([128, HIDDEN], F32, tag="biaso")
                    nc.sync.dma_start(bias_o_sb[:], d_bias_o.ap())
                wek = []
                for k in range(KT_E):
                    wt = p0w.tile([128, HIDDEN], BF16, tag=f"p0we{k}", name=f"p0we{k}")
                    nc.sync.dma_start(wt[:], d_We.ap()[k * 128:(k + 1) * 128, :])
                    wek.append(wt)
                for mt in range(T * BL // 128):
                    ek = []
                    for k in range(KT_E):
                        ekt = p0e.tile([128, 128], BF16, tag="p0ek")
                        nc.sync.dma_start(ekt[:], d_embT.ap()[k * 128:(k + 1) * 128,
                                                              mt * 128:(mt + 1) * 128])
                        ek.append(ekt)
                    xop0 = spool.tile([128, HIDDEN], F32, tag="p0out")
                    for nt in range(2):
                        ps = p0p.tile([128, 512], F32, tag="p0ps")
                        for k in range(KT_E):
                            nc.tensor.matmul(
                                ps[:], ek[k][:],
                                wek[k][:, nt * 512:(nt + 1) * 512],
                                start=(k == 0), stop=(k == KT_E - 1))
                        # chunk c = 4*nt + j  ->  permuted col j*256 + nt*128
                        for j in range(4):
                            dst = xop0[:, j * 256 + nt * 128:j * 256 + nt * 128 + 128]
                            src = ps[:, j * 128:(j + 1) * 128]
                            if bias_o_sb is not None:
                                nc.vector.tensor_add(
                                    dst, src,
                                    bias_o_sb[:, j * 256 + nt * 128:j * 256 + nt * 128 + 128])
                            elif j % 2 == 0:
                                nc.scalar.copy(dst, src)
                            else:
                                nc.vector.tensor_copy(dst, src)
                    nc.sync.dma_start(s_xop.ap()[mt * 128:(mt + 1) * 128, :], xop0[:])

                # zero the pad regions of deterT_all
                zpad = spool.tile([128, 128], BF16, tag="zpad")
                nc.vector.memset(zpad[:], 0.0)
                for (_k, r0, r1, ts) in heads:
                    pr0 = r0 + len(ts) * BL
                    if pr0 < r1:
                        for kk in range(8):
                            nc.sync.dma_start(s_dTa.ap()[kk, :, pr0:r1], zpad[:, 0:r1 - pr0])

            # ---------- scan ----------
            with tc.tile_pool(name="gpool", bufs=2, space="PSUM") as gpool, \
                 tc.tile_pool(name="xpool", bufs=1, space="PSUM") as xpool, \
                 tc.tile_pool(name="opool", bufs=1, space="PSUM") as opool, \
                 tc.tile_pool(name="dpool", bufs=1, space="PSUM") as dpool, \
                 tc.tile_pool(name="tpool", bufs=2, space="PSUM") as tpool, \
                 tc.tile_pool(name="p2p", bufs=1, space="PSUM") as p2p, \
                 tc.tile_pool(name="p2w", bufs=2) as p2w, \
                 tc.tile_pool(name="p2d", bufs=10) as p2d:

                def rsqrt_dve(st, ps_tile, pcol, outcol, qsb, magic_w):
                    """copy-based m/q; m^2 via vector TT (known good)."""
                    nc.vector.tensor_copy(st[:, 4:6], ps_tile[:, pcol:pcol + 2])
                    m, q = st[:, 4:5], st[:, 5:6]
                    nc.vector.tensor_tensor(st[:, 1:2], m, m, ALU.mult)
                    v = st[:, 0:1]
                    nc.vector.scalar_tensor_tensor(v, q, 1e-5, st[:, 1:2],
                                                   ALU.add, ALU.subtract)
                    yi = st[:, outcol:outcol + 1]
                    nc.vector.tensor_scalar(st[:, 2:3].bitcast(I32), v.bitcast(I32),
                                            1, None, ALU.arith_shift_right)
                    nc.vector.scalar_tensor_tensor(yi.bitcast(I32), magic[:], 1,
                                                   st[:, 2:3].bitcast(I32),
                                                   ALU.mult, ALU.subtract)
                    h = st[:, 1:2]
                    for _ in range(NEWTON_ITERS):
                        nc.vector.tensor_tensor(h, yi, yi, ALU.mult)
                        nc.vector.tensor_tensor(h, h, v, ALU.mult)
                        nc.vector.tensor_scalar(h, h, -0.5, 1.5, ALU.mult, ALU.add)
                        nc.vector.tensor_tensor(yi, yi, h, ALU.mult)
                    return m, yi

                magic = wpool.tile([128, 1], I32, tag="magic")
            nc.vector.memset(magic[:], MAGIC)
            magic_g = wpool.tile([128, 1], I32, tag="magic_g")
            nc.vector.memset(magic_g[:], MAGIC_G)
            magic_h = wpool.tile([128, 1], I32, tag="magic_h")
            nc.vector.memset(magic_h[:], MAGIC_H)
            if not affine_trivial:
                gbn_g = wpool.tile([128, 4 * 384], F32, tag="gg")
                nc.sync.dma_start(gbn_g[:], d_gbn_g.ap())
                gbn_o = wpool.tile([128, 2 * 256], F32, tag="go")
                nc.sync.dma_start(gbn_o[:], d_gbn_o.ap())
                gbn_i = wpool.tile([128, 2 * 256], F32, tag="gi")
                nc.sync.dma_start(gbn_i[:], d_gbn_i.ap())
                bod = wpool.tile([2 * STOCH, BL], F32, tag="bod")
                nc.sync.dma_start(bod[:], d_bod.ap())

            # ---------- phase 0: xop = embed @ We (+bias), column-permuted ----------
            with tc.tile_pool(name="p0w", bufs=1) as p0w, \
                 tc.tile_pool(name="p0e", bufs=13) as p0e, \
                 tc.tile_pool(name="p0p", bufs=2, space="PSUM") as p0p:
                bias_o_sb = None
                if not affine_trivial:
                    bias_o_sb = p0w.tile([128, HIDDEN], F32, tag="biaso")
                    nc.sync.dma_start(bias_o_sb[:], d_bias_o.ap())
                wek = []
                for k in range(KT_E):
                    wt = p0w.tile([128, HIDDEN], BF16, tag=f"p0we{k}", name=f"p0we{k}")
                    nc.sync.dma_start(wt[:], d_We.ap()[k * 128:(k + 1) * 128, :])
                    wek.append(wt)
                for mt in range(T * BL // 128):
                    ek = []
                    for k in range(KT_E):
                        ekt = p0e.tile([128, 128], BF16, tag="p0ek")
                        nc.sync.dma_start(ekt[:], d_embT.ap()[k * 128:(k + 1) * 128,
                                                              mt * 128:(mt + 1) * 128])
                        ek.append(ekt)
                    xop0 = spool.tile([128, HIDDEN], F32, tag="p0out")
                    for nt in range(2):
                        ps = p0p.tile([128, 512], F32, tag="p0ps")
                        for k in range(KT_E):
                            nc.tensor.matmul(
                                ps[:], ek[k][:],
                                wek[k][:, nt * 512:(nt + 1) * 512],
                                start=(k == 0), stop=(k == KT_E - 1))
                        # chunk c = 4*nt + j  ->  permuted col j*256 + nt*128
                        for j in range(4):
                            dst = xop0[:, j * 256 + nt * 128:j * 256 + nt * 128 + 128]
                            src = ps[:, j * 128:(j + 1) * 128]
                            if bias_o_sb is not None:
                                nc.vector.tensor_add(
                                    dst, src,
                                    bias_o_sb[:, j * 256 + nt * 128:j * 256 + nt * 128 + 128])
                            elif j % 2 == 0:
                                nc.scalar.copy(dst, src)
                            else:
                                nc.vector.tensor_copy(dst, src)
                    nc.sync.dma_start(s_xop.ap()[mt * 128:(mt + 1) * 128, :], xop0[:])

                # zero the pad regions of deterT_all
                zpad = spool.tile([128, 128], BF16, tag="zpad")
                nc.vector.memset(zpad[:], 0.0)
                for (_k, r0, r1, ts) in heads:
                    pr0 = r0 + len(ts) * BL
                    if pr0 < r1:
                        for kk in range(8):
                            nc.sync.dma_start(s_dTa.ap()[kk, :, pr0:r1], zpad[:, 0:r1 - pr0])

            # ---------- scan ----------
            with tc.tile_pool(name="gpool", bufs=2, space="PSUM") as gpool, \
                 tc.tile_pool(name="xpool", bufs=1, space="PSUM") as xpool, \
                 tc.tile_pool(name="opool", bufs=1, space="PSUM") as opool, \
                 tc.tile_pool(name="dpool", bufs=1, space="PSUM") as dpool, \
                 tc.tile_pool(name="tpool", bufs=2, space="PSUM") as tpool, \
                 tc.tile_pool(name="p2p", bufs=1, space="PSUM") as p2p, \
                 tc.tile_pool(name="p2w", bufs=2) as p2w, \
                 tc.tile_pool(name="p2d", bufs=10) as p2d:

                def rsqrt_dve(st, ps_tile, pcol, outcol, qsb, magic_w):
                    """st[:, outcol] = 1/sqrt(q - m^2 + eps), (m, q) = psum tile
                    cols (pcol, pcol+1). m^2 on scalar (psum read) in parallel;
                    v in sbuf; bit-trick seed + Newton. Returns (m_psum, rstd)."""
                    m = ps_tile[:, pcol:pcol + 1]
                    q = ps_tile[:, pcol + 1:pcol + 2]
                    nc.scalar.square(st[:, 1:2], m)
                    v = st[:, 0:1]
                    nc.vector.scalar_tensor_tensor(v, q, 1e-5, st[:, 1:2],
                                                   ALU.add, ALU.subtract)
                    yi = st[:, outcol:outcol + 1]
                    nc.vector.tensor_scalar(st[:, 2:3].bitcast(I32), v.bitcast(I32),
                                            1, None, ALU.arith_shift_right)
                    nc.vector.scalar_tensor_tensor(yi.bitcast(I32), magic[:], 1,
                                                   st[:, 2:3].bitcast(I32),
                                                   ALU.mult, ALU.subtract)
                    h = st[:, 1:2]
                    for _ in range(NEWTON_ITERS):
                        nc.vector.tensor_tensor(h, yi, yi, ALU.mult)
                        nc.vector.tensor_tensor(h, h, v, ALU.mult)
                        nc.vector.tensor_scalar(h, h, -0.5, 1.5, ALU.mult, ALU.add)
                        nc.vector.tensor_tensor(yi, yi, h, ALU.mult)
                    return m, yi

                magic = wpool.tile([128, 1], I32, tag="magic")
            nc.vector.memset(magic[:], MAGIC)
            magic_g = wpool.tile([128, 1], I32, tag="magic_g")
            nc.vector.memset(magic_g[:], MAGIC_G)
            magic_h = wpool.tile([128, 1], I32, tag="magic_h")
            nc.vector.memset(magic_h[:], MAGIC_H)
            if not affine_trivial:
                gbn_g = wpool.tile([128, 4 * 384], F32, tag="gg")
                nc.sync.dma_start(gbn_g[:], d_gbn_g.ap())
                gbn_o = wpool.tile([128, 2 * 256], F32, tag="go")
                nc.sync.dma_start(gbn_o[:], d_gbn_o.ap())
                gbn_i = wpool.tile([128, 2 * 256], F32, tag="gi")
                nc.sync.dma_start(gbn_i[:], d_gbn_i.ap())
                bod = wpool.tile([2 * STOCH, BL], F32, tag="bod")
                nc.sync.dma_start(bod[:], d_bod.ap())

            # ---------- phase 0: xop = embed @ We (+bias), column-permuted ----------
            with tc.tile_pool(name="p0w", bufs=1) as p0w, \
                 tc.tile_pool(name="p0e", bufs=13) as p0e, \
                 tc.tile_pool(name="p0p", bufs=2, space="PSUM") as p0p:
                bias_o_sb = None
                if not affine_trivial:
                    bias_o_sb = p0w.tile([128, HIDDEN], F32, tag="biaso")
                    nc.sync.dma_start(bias_o_sb[:], d_bias_o.ap())
                wek = []
                for k in range(KT_E):
                    wt = p0w.tile([128, HIDDEN], BF16, tag=f"p0we{k}", name=f"p0we{k}")
                    nc.sync.dma_start(wt[:], d_We.ap()[k * 128:(k + 1) * 128, :])
                    wek.append(wt)
                for mt in range(T * BL // 128):
                    ek = []
                    for k in range(KT_E):
                        ekt = p0e.tile([128, 128], BF16, tag="p0ek")
                        nc.sync.dma_start(ekt[:], d_embT.ap()[k * 128:(k + 1) * 128,
                                                              mt * 128:(mt + 1) * 128])
                        ek.append(ekt)
                    xop0 = spool.tile([128, HIDDEN], F32, tag="p0out")
                    for nt in range(2):
                        ps = p0p.tile([128, 512], F32, tag="p0ps")
                        for k in range(KT_E):
                            nc.tensor.matmul(
                                ps[:], ek[k][:],
                                wek[k][:, nt * 512:(nt + 1) * 512],
                                start=(k == 0), stop=(k == KT_E - 1))
                        # chunk c = 4*nt + j  ->  permuted col j*256 + nt*128
                        for j in range(4):
                            dst = xop0[:, j * 256 + nt * 128:j * 256 + nt * 128 + 128]
                            src = ps[:, j * 128:(j + 1) * 128]
                            if bias_o_sb is not None:
                                nc.vector.tensor_add(
                                    dst, src,
                                    bias_o_sb[:, j * 256 + nt * 128:j * 256 + nt * 128 + 128])
                            elif j % 2 == 0:
                                nc.scalar.copy(dst, src)
                            else:
                                nc.vector.tensor_copy(dst, src)
                    nc.sync.dma_start(s_xop.ap()[mt * 128:(mt + 1) * 128, :], xop0[:])

                # zero the pad regions of deterT_all
                zpad = spool.tile([128, 128], BF16, tag="zpad")
                nc.vector.memset(zpad[:], 0.0)
                for (_k, r0, r1, ts) in heads:
                    pr0 = r0 + len(ts) * BL
                    if pr0 < r1:
                        for kk in range(8):
                            nc.sync.dma_start(s_dTa.ap()[kk, :, pr0:r1], zpad[:, 0:r1 - pr0])

            # ---------- scan ----------
            with tc.tile_pool(name="gpool", bufs=2, space="PSUM") as gpool, \
                 tc.tile_pool(name="xpool", bufs=1, space="PSUM") as xpool, \
                 tc.tile_pool(name="opool", bufs=1, space="PSUM") as opool, \
                 tc.tile_pool(name="dpool", bufs=1, space="PSUM") as dpool, \
                 tc.tile_pool(name="tpool", bufs=2, space="PSUM") as tpool, \
                 tc.tile_pool(name="p2p", bufs=1, space="PSUM") as p2p, \
                 tc.tile_pool(name="p2w", bufs=2) as p2w, \
                 tc.tile_pool(name="p2d", bufs=10) as p2d:

                def rsqrt_dve(st, ps_tile, pcol, outcol, qsb, magic_w):
                    """st[:, outcol] = 1/sqrt(q - m^2 + eps); copy-based."""
                    nc.vector.tensor_copy(st[:, 4:6], ps_tile[:, pcol:pcol + 2])
                    m, q = st[:, 4:5], st[:, 5:6]
                    v = st[:, 0:1]
                    nc.vector.tensor_tensor(st[:, 1:2], m, m, ALU.mult)
                    nc.vector.scalar_tensor_tensor(v, q, 1e-5, st[:, 1:2],
                                                   ALU.add, ALU.subtract)
                    yi = st[:, outcol:outcol + 1]
                    nc.vector.tensor_scalar(st[:, 2:3].bitcast(I32), v.bitcast(I32),
                                            1, None, ALU.arith_shift_right)
                    nc.vector.scalar_tensor_tensor(yi.bitcast(I32), magic[:], 1,
                                                   st[:, 2:3].bitcast(I32),
                                                   ALU.mult, ALU.subtract)
                    h = st[:, 1:2]
                    for _ in range(NEWTON_ITERS):
                        nc.vector.tensor_tensor(h, yi, yi, ALU.mult)
                        nc.vector.tensor_tensor(h, h, v, ALU.mult)
                        nc.vector.tensor_scalar(h, h, -0.5, 1.5, ALU.mult, ALU.add)
                        nc.vector.tensor_tensor(yi, yi, h, ALU.mult)
                    return m, yi

                def rsqrt_sb(st, m, q, outcol, eps_done=False):
                    """st[:, outcol] = 1/sqrt(q - m^2 + eps); all-sbuf variant."""
                    v = st[:, 0:1]
                    nc.vector.tensor_tensor(st[:, 1:2], m, m, ALU.mult)
                    nc.vector.scalar_tensor_tensor(v, q, 1e-5, st[:, 1:2],
                                                   ALU.add, ALU.subtract)
                    yi = st[:, outcol:outcol + 1]
                    nc.vector.tensor_scalar(st[:, 2:3].bitcast(I32), v.bitcast(I32),
                                            1, None, ALU.arith_shift_right)
                    nc.vector.scalar_tensor_tensor(yi.bitcast(I32), magic[:], 1,
                                                   st[:, 2:3].bitcast(I32),
                                                   ALU.mult, ALU.subtract)
                    h = st[:, 1:2]
                    for _ in range(2):
                        nc.vector.tensor_tensor(h, yi, yi, ALU.mult)
                        nc.vector.tensor_tensor(h, h, v, ALU.mult)
                        nc.vector.tensor_scalar(h, h, -0.5, 1.5, ALU.mult, ALU.add)
                        nc.vector.tensor_tensor(yi, yi, h, ALU.mult)
                    return yi

                def emit_phase2_group(hk, mt):
                    """Ensemble prior head hk for row-group mt (filler work —
                    interleaved into scan idle). Raw pstd out (host softplus)."""
                    Wio = p2w.tile([128, 8 * HIDDEN], BF16, tag="Wio",
                                   name=f"Wio_{hk}_{mt}")
                    for kk in range(8):
                        nc.sync.dma_start(Wio[:, kk * 1024:(kk + 1) * 1024],
                                          d_Wio.ap()[hk, kk * 128:(kk + 1) * 128, :])
                    Wids = p2w.tile([128, 8 * 2 * STOCH], BF16, tag="Wids",
                                    name=f"Wids_{hk}_{mt}")
                    for kk in range(8):
                        nc.sync.dma_start(Wids[:, kk * 64:(kk + 1) * 64],
                                          d_Wids.ap()[hk, kk * 128:(kk + 1) * 128, :])
                    if not affine_trivial:
                        gbn_e = p2w.tile([128, 2 * HIDDEN], F32, tag="gbnE",
                                         name=f"gbnE_{hk}_{mt}")
                        nc.sync.dma_start(gbn_e[:], d_gbn_e.ap()[hk])
                        bias_e = p2w.tile([128, HIDDEN], F32, tag="biasE",
                                          name=f"biasE_{hk}_{mt}")
                        nc.sync.dma_start(bias_e[:], d_bias_e.ap()[hk])
                        bids = p2w.tile([128, 2 * STOCH], F32, tag="bidsE",
                                        name=f"bidsE_{hk}_{mt}")
                        nc.sync.dma_start(bids[:], d_bids.ap()[hk])
                    dk = []
                    for kk in range(8):
                        dkt = p2d.tile([128, 128], BF16, tag="p2dk",
                                       name=f"p2dk_{hk}_{mt}_{kk}")
                        nc.sync.dma_start(dkt[:],
                                          s_dTa.ap()[kk, :, mt * 128:(mt + 1) * 128])
                        dk.append(dkt)
                    z_e = spool.tile([128, HIDDEN], F32, tag="z_e")
                    estats = spool.tile([128, 8], F32, tag="estats")
                    for nt in range(2):
                        ps = p2p.tile([128, 512], F32, tag="p2ps")
                        for kk in range(8):
                            nc.tensor.matmul(
                                ps[:], dk[kk][:],
                                Wio[:, kk * 1024 + nt * 512: kk * 1024 + nt * 512 + 512],
                                start=(kk == 0), stop=(kk == 7))
                        if not affine_trivial:
                            nc.vector.tensor_add(z_e[:, nt * 512:(nt + 1) * 512], ps[:],
                                                 bias_e[:, nt * 512:(nt + 1) * 512])
                        else:
                            nc.scalar.copy(z_e[:, nt * 512:(nt + 1) * 512], ps[:])
                        nc.vector.reduce_sum(estats[:, 4 + nt:5 + nt],
                                             z_e[:, nt * 512:(nt + 1) * 512], axis=AX)
                        sqe = spool.tile([128, 512], F32, tag="sqt2")
                        nc.scalar.activation(sqe[:], z_e[:, nt * 512:(nt + 1) * 512],
                                             AF.Square,
                                             accum_out=estats[:, 6 + nt:7 + nt])
                    se = spool.tile([128, 8], F32, tag="lnst")
                    nc.vector.tensor_add(se[:, 4:5], estats[:, 4:5], estats[:, 5:6])
                    nc.vector.tensor_add(se[:, 5:6], estats[:, 6:7], estats[:, 7:8])
                    nc.vector.tensor_scalar_mul(se[:, 4:5], se[:, 4:5], 1.0 / HIDDEN)
                    nc.vector.tensor_scalar_mul(se[:, 5:6], se[:, 5:6], 1.0 / HIDDEN)
                    rstd_e = rsqrt_sb(se, se[:, 4:5], se[:, 5:6], 6)
                    nmr_e = se[:, 7:8]
                    nc.vector.scalar_tensor_tensor(nmr_e, se[:, 4:5], -1.0, rstd_e,
                                                   ALU.mult, ALU.mult)
                    hsb = spool.tile([128, HIDDEN], BF16, tag="hsb")
                    if affine_trivial:
                        ee = spool.tile([128, HIDDEN], F32, tag="ee")
                        nc.scalar.activation(ee[:], z_e[:], AF.Exp,
                                             bias=nmr_e, scale=rstd_e)
                        xre = spool.tile([128, HIDDEN], F32, tag="xre")
                        nc.vector.tensor_scalar(xre[:], z_e[:], rstd_e, nmr_e,
                                                ALU.mult, ALU.add)
                        nc.vector.tensor_scalar_max(xre[:], xre[:], 0.0)
                        nc.vector.scalar_tensor_tensor(hsb[:], ee[:], -1.0, xre[:],
                                                       ALU.add, ALU.min)
                    else:
                        eln = z_e
                        nc.vector.tensor_scalar(eln[:], z_e[:], rstd_e, nmr_e,
                                                ALU.mult, ALU.add)
                        nc.vector.tensor_mul(eln[:], eln[:], gbn_e[:, 0:HIDDEN])
                        nc.vector.tensor_add(eln[:], eln[:], gbn_e[:, HIDDEN:])
                        ee = spool.tile([128, HIDDEN], F32, tag="ee")
                        nc.scalar.activation(ee[:], eln[:], AF.Exp)
                        xre = spool.tile([128, HIDDEN], F32, tag="xre")
                        nc.vector.tensor_scalar_max(xre[:], eln[:], 0.0)
                        nc.vector.scalar_tensor_tensor(hsb[:], ee[:], -1.0, xre[:],
                                                       ALU.add, ALU.min)
                    pse_t = dpool.tile([128, 96], F32, tag="pd", name="pse_t")
                    pse = pse_t[:, 32:96]
                    for kk in range(8):
                        tpe = tpool.tile([128, 128], BF16, tag="tp",
                                         name=f"tpe_{hk}_{mt}_{kk}")
                        nc.tensor.transpose(tpe[:], hsb[:, kk * 128:(kk + 1) * 128],
                                            eye128b[:])
                        hst = spool.tile([128, 128], BF16, tag="hst")
                        nc.scalar.copy(hst[:], tpe[:])
                        nc.tensor.matmul(pse, hst[:],
                                         Wids[:, kk * 64:(kk + 1) * 64],
                                         start=(kk == 0), stop=(kk == 7))
                    ed = spool.tile([128, 2 * STOCH], F32, tag="ed")
                    nc.vector.tensor_copy(ed[:], pse)
                    if not affine_trivial:
                        nc.vector.tensor_add(ed[:], ed[:], bids[:])
                    for t2 in gts_by_group[(hk, mt)]:
                        lr = row_ofs[t2] - mt * 128
                        nc.sync.dma_start(o_ens.ap()[:, t2, :], ed[lr:lr + BL, :])

                deter = spool2.tile([128, 256], F32, tag="deterN")
                nc.vector.memset(deter[:], 0.0)
                dTm = [spool2.tile([128, 128], BF16, tag=f"dTm{i}",
                                   name=f"dTm_init{i}") for i in range(2)]
                nc.vector.memset(dTm[0][:], 0.0)
                nc.vector.memset(dTm[1][:], 0.0)
                in45T = spool2.tile([45, BL], BF16, tag="in45T")
                nc.vector.memset(in45T[0:STOCH, :], 0.0)
                nc.sync.dma_start(in45T[STOCH:, :], d_am1T.ap()[:, 0:BL])
                xop_f = spool2.tile([128, 256], F32, tag="xop_f")
                for s in range(4):
                    nc.sync.dma_start(xop_f[32 * s:32 * s + 16, :],
                                      s_xop.ap()[0:BL, s * 256:(s + 1) * 256])

                for t in range(T):
                    # ---- img MLP quad: x = elu(ln(in45 @ Wimg)) ----
                    px = xpool.tile([128, 264], F32, tag="px")
                    if t == 0:
                        # one-time: zero stale PSUM (possibly Inf/NaN) so the
                        # never-written garbage strips can't poison accums /
                        # selrep contractions
                        nc.vector.memset(px[:], 0.0)
                    for s in range(4):
                        nc.tensor.matmul(px[32 * s:32 * s + 16, 0:257],
                                         in45T[:], Wimg[:, s * 257:(s + 1) * 257],
                                         start=True, stop=True,
                                         tile_position=(0, 32 * s))

                    # ---- GRU quads, d-part first (only needs deterT(t-1)) ----
                    pg = [gpool.tile([128, 388], F32, tag="pg", name=f"pg{i}")
                          for i in range(2)]  # noqa
                    if t < 2:  # cover both rotation buffers of the pool
                        for i in range(2):
                            nc.vector.memset(pg[i][:], 0.0)

                    def gru_emit(kk, blk, scol, first, last):
                        for i in range(2):
                            for s4 in range(4):
                                c = 4 * i + s4
                                nc.tensor.matmul(
                                    pg[i][32 * s4:32 * s4 + 16, 0:385],
                                    blk[:, scol:scol + 16],
                                    Wg[:, (kk * 8 + c) * 385:(kk * 8 + c + 1) * 385],
                                    start=first, stop=last,
                                    tile_position=(0, 32 * s4))

                    for k in range(8):  # d-part: Wg k-tiles 8..15
                        gru_emit(8 + k, dTm[k // 4], 32 * (k % 4), k == 0, False)

                    # ---- img postprocess (overlaps GRU-d on PE) ----
                    st_i = spool2.tile([128, 12], F32, tag="st_i")
                    nc.vector.tensor_copy(st_i[:, 8:9], px[:, 256:257])
                    sq_i = spool.tile([128, 256], F32, tag="sqscratch", name="sq_i")
                    nc.scalar.activation(sq_i[:], px[:, 0:256], AF.Square,
                                         accum_out=st_i[:, 9:10])
                    nc.tensor.matmul(px[:, 260:262], selh[:], st_i[:, 8:10],
                                     start=True, stop=True)
                    m_i, rstd_i = rsqrt_dve(st_i, px, 260, 6,
                                            st_i[:, 9:10], magic_h[:])
                    nmr_i = st_i[:, 7:8]
                    nc.vector.scalar_tensor_tensor(nmr_i, m_i, -1.0, rstd_i,
                                                   ALU.mult, ALU.mult)
                    xb = spool.tile([128, 256], BF16, tag="xb")
                    if affine_trivial:
                        # exp(apply) on scalar || relu(apply) on vector
                        ex = spool.tile([128, 256], F32, tag="ex")
                        nc.scalar.activation(ex[:], px[:, 0:256], AF.Exp,
                                             bias=nmr_i, scale=rstd_i)
                        xr = spool.tile([128, 256], BF16, tag="xr")
                        nc.vector.tensor_scalar(xr[:], px[:, 0:256], rstd_i,
                                                nmr_i, ALU.mult, ALU.add)
                        nc.vector.tensor_scalar_max(xr[:], xr[:], 0.0)
                        nc.vector.scalar_tensor_tensor(xb[:], ex[:], -1.0, xr[:],
                                                       ALU.add, ALU.min)
                    else:
                        xh = spool.tile([128, 256], F32, tag="xh")
                        nc.scalar.activation(xh[:], px[:, 0:256], AF.Identity,
                                             bias=nmr_i, scale=rstd_i)
                        nc.vector.tensor_mul(xh[:], xh[:], gbn_i[:, 0:256])
                        nc.vector.tensor_add(xh[:], xh[:], gbn_i[:, 256:512])
                        ex = spool.tile([128, 256], F32, tag="ex")
                        nc.scalar.activation(ex[:], xh[:], AF.Exp)
                        xr = spool.tile([128, 256], F32, tag="xr")
                        nc.vector.tensor_scalar_max(xr[:], xh[:], 0.0)
                        nc.vector.scalar_tensor_tensor(xb[:], ex[:], -1.0, xr[:],
                                                       ALU.add, ALU.min)
                    xT = []
                    for i in range(2):
                        tp = tpool.tile([128, 128], BF16, tag="tp", name=f"tpx{i}")
                        nc.tensor.transpose(tp[:], xb[:, i * 128:(i + 1) * 128],
                                            eye128b[:])
                        xt = spool.tile([128, 128], BF16, tag=f"xT{i}")
                        nc.vector.tensor_copy(xt[:], tp[:])
                        xT.append(xt)

                    # ---- GRU x-part quads ----
                    for k in range(8):
                        gru_emit(k, xT[k // 4], 32 * (k % 4), False, k == 7)

                    # phase-2 filler emitted here: behind this step's critical
                    # img chain in queue order, ahead of the gates idle window
                    for (hk_, mt_) in group_ready.get(t - 1, []):
                        emit_phase2_group(hk_, mt_)

                    # ---- GRU stats ----
                    st_g = spool2.tile([128, 12], F32, tag="st_g")
                    nc.vector.tensor_copy(st_g[:, 8:9], pg[0][:, 384:385])
                    nc.vector.tensor_copy(st_g[:, 9:10], pg[1][:, 384:385])
                    sq_g = spool.tile([128, 384], F32, tag="sqg")
                    nc.scalar.activation(sq_g[:], pg[0][:, 0:384], AF.Square,
                                         accum_out=st_g[:, 10:11])
                    zg1 = spool.tile([128, 384], F32, tag="zg1")
                    nc.vector.tensor_copy(zg1[:], pg[1][:, 0:384])
                    sq_g2 = spool.tile([128, 384], F32, tag="sqg2")
                    nc.vector.scalar_tensor_tensor(sq_g2[:], zg1[:], 1.0, zg1[:],
                                                   ALU.mult, ALU.mult,
                                                   accum_out=st_g[:, 11:12])
                    st_in = spool2.tile([128, 2], F32, tag="st_in")
                    nc.vector.tensor_add(st_in[:, 0:1], st_g[:, 8:9], st_g[:, 9:10])
                    nc.vector.tensor_add(st_in[:, 1:2], st_g[:, 10:11], st_g[:, 11:12])
                    nc.tensor.matmul(pg[0][:, 385:387], selg[:], st_in[:],
                                     start=True, stop=True)
                    m_g, rstd_g = rsqrt_dve(st_g, pg[0], 385, 6,
                                            st_in[:, 1:2], magic_g[:])
                    mr_g = st_g[:, 7:8]
                    nc.vector.tensor_tensor(mr_g, m_g, rstd_g, ALU.mult)
                    gs = spool2.tile([128, 4], F32, tag="gs")
                    nc.vector.tensor_scalar_mul(gs[:, 0:1], rstd_g, 0.5)
                    nc.vector.tensor_scalar_mul(gs[:, 1:2], mr_g, -0.5)
                    nc.vector.tensor_scalar(gs[:, 2:3], mr_g, -0.5,
                                            0.5 * UPDATE_BIAS, ALU.mult, ALU.add)

                    if not affine_trivial:
                        zn = [spool.tile([128, 384], F32, tag=f"zn{i}",
                                         name=f"zn_{i}") for i in range(2)]
                        for i in range(2):
                            nc.vector.tensor_scalar(zn[i][:], pg[i][:, 0:384],
                                                    rstd_g, mr_g,
                                                    ALU.mult, ALU.subtract)
                            nc.vector.tensor_mul(zn[i][:], zn[i][:],
                                                 gbn_g[:, (2 * i) * 384:(2 * i + 1) * 384])
                            nc.vector.tensor_add(zn[i][:], zn[i][:],
                                                 gbn_g[:, (2 * i + 1) * 384:(2 * i + 2) * 384])
                        gate_src = [zn[0][:], zn[1][:]]
                    else:
                        gate_src = [pg[0][:], pg[1][:]]

                    # ---- gates (sigmoid via tanh) ----
                    th_r = spool.tile([128, 256], F32, tag="th_r")
                    cn = spool.tile([128, 256], F32, tag="cn")
                    cand = spool.tile([128, 256], F32, tag="cand")
                    th_u = spool.tile([128, 256], F32, tag="th_u")
                    for i in range(2):
                        sl = slice(i * 128, (i + 1) * 128)
                        src = gate_src[i]
                        if affine_trivial:
                            nc.scalar.activation(th_r[:, sl], src[:, 0:128], AF.Tanh,
                                                 bias=gs[:, 1:2], scale=gs[:, 0:1])
                            nc.vector.tensor_scalar(cn[:, sl], src[:, 128:256],
                                                    rstd_g, mr_g,
                                                    ALU.mult, ALU.subtract)
                            nc.scalar.activation(th_u[:, sl], src[:, 256:384], AF.Tanh,
                                                 bias=gs[:, 2:3], scale=gs[:, 0:1])
                        else:
                            nc.scalar.activation(th_r[:, sl], src[:, 0:128], AF.Tanh,
                                                 scale=0.5)
                            nc.vector.tensor_copy(cn[:, sl], src[:, 128:256])
                            nc.scalar.activation(th_u[:, sl], src[:, 256:384], AF.Tanh,
                                                 bias=0.5 * UPDATE_BIAS, scale=0.5)
                    nc.vector.scalar_tensor_tensor(cn[:], th_r[:], 1.0,
                                                   cn[:], ALU.add, ALU.mult)
                    nc.vector.tensor_scalar(th_u[:], th_u[:], 0.5, 0.5,
                                            ALU.mult, ALU.add)
                    nc.scalar.activation(cand[:], cn[:], AF.Tanh, scale=0.5)

                    prev = deter
                    deter = spool2.tile([128, 256], F32, tag="deterN")
                    m_t = maskF[:, t:t + 1]
                    t1 = cn  # reuse
                    nc.vector.scalar_tensor_tensor(t1[:], prev[:], m_t, cand[:],
                                                   ALU.mult, ALU.subtract)
                    nc.vector.tensor_mul(t1[:], t1[:], th_u[:])
                    nc.vector.scalar_tensor_tensor(deter[:], prev[:], m_t, t1[:],
                                                   ALU.mult, ALU.subtract)
                    nc.sync.dma_start(o_detF.ap()[:, t, :], deter[:])

                    # deterT blocks: unmasked (obs, phase2) + masked (next GRU-d)
                    db = spool.tile([128, 256], BF16, tag="db")
                    nc.vector.tensor_copy(db[:], deter[:])
                    dbm = spool.tile([128, 256], BF16, tag="dbm")
                    nc.scalar.activation(dbm[:], deter[:], AF.Copy,
                                         scale=maskF[:, t + 1:t + 2])
                    dTu = []
                    dTm = []
                    for i in range(2):
                        tpu = tpool.tile([128, 128], BF16, tag="tp", name=f"tpdu{i}")
                        nc.tensor.transpose(tpu[:], db[:, i * 128:(i + 1) * 128],
                                            eye128b[:])
                        du = spool2.tile([128, 128], BF16, tag=f"dTu{i}")
                        if i == 0:
                            nc.vector.tensor_copy(du[:], tpu[:])
                        else:
                            nc.scalar.copy(du[:], tpu[:])
                        dTu.append(du)
                        tpm = tpool.tile([128, 128], BF16, tag="tp", name=f"tpdm{i}")
                        nc.tensor.transpose(tpm[:], dbm[:, i * 128:(i + 1) * 128],
                                            eye128b[:])
                        dm_ = spool2.tile([128, 128], BF16, tag=f"dTm{i}")
                        nc.scalar.copy(dm_[:], tpm[:])
                        dTm.append(dm_)
                    # feed phase 2 (unmasked)
                    for c in range(8):
                        eng = nc.gpsimd if c % 2 == 0 else nc.sync
                        eng.dma_start(s_dTa.ap()[c, :, row_ofs[t]:row_ofs[t] + BL],
                                      dTu[c // 4][:, 32 * (c % 4):32 * (c % 4) + 16])

                    # ---- obs posterior quads ----
                    po = opool.tile([128, 260], F32, tag="po")
                    if t == 0:
                        nc.vector.memset(po[:], 0.0)
                    for k in range(8):
                        blk, s = dTu[k // 4], k % 4
                        for s4 in range(4):
                            nc.tensor.matmul(
                                po[32 * s4:32 * s4 + 16, 0:256],
                                blk[:, 32 * s:32 * s + 16],
                                Wd[:, (k * 4 + s4) * 256:(k * 4 + s4 + 1) * 256],
                                start=(k == 0), stop=(k == 7),
                                tile_position=(0, 32 * s4))
                    z_o = spool.tile([128, 256], F32, tag="z_o")
                    st_o = spool2.tile([128, 12], F32, tag="st_o")
                    nc.vector.scalar_tensor_tensor(z_o[:], po[:, 0:256], 1.0,
                                                   xop_f[:], ALU.mult, ALU.add,
                                                   accum_out=st_o[:, 8:9])
                    sq_o = spool.tile([128, 256], F32, tag="sqscratch", name="sq_o")
                    nc.vector.scalar_tensor_tensor(sq_o[:], z_o[:], 1.0, z_o[:],
                                                   ALU.mult, ALU.mult,
                                                   accum_out=st_o[:, 9:10])
                    nc.tensor.matmul(po[:, 256:258], selh[:], st_o[:, 8:10],
                                     start=True, stop=True)
                    m_o, rstd_o = rsqrt_dve(st_o, po, 256, 6,
                                            st_o[:, 9:10], magic_h[:])
                    nmr_o = st_o[:, 7:8]
                    nc.vector.scalar_tensor_tensor(nmr_o, m_o, -1.0, rstd_o,
                                                   ALU.mult, ALU.mult)
                    xob = spool.tile([128, 256], BF16, tag="xob")
                    if affine_trivial:
                        eo = spool.tile([128, 256], F32, tag="ex", name="eo")
                        nc.scalar.activation(eo[:], z_o[:], AF.Exp,
                                             bias=nmr_o, scale=rstd_o)
                        xro = spool.tile([128, 256], BF16, tag="xr", name="xro")
                        nc.vector.tensor_scalar(xro[:], z_o[:], rstd_o,
                                                nmr_o, ALU.mult, ALU.add)
                        nc.vector.tensor_scalar_max(xro[:], xro[:], 0.0)
                        nc.vector.scalar_tensor_tensor(xob[:], eo[:], -1.0, xro[:],
                                                       ALU.add, ALU.min)
                    else:
                        oh = z_o
                        nc.scalar.activation(oh[:], z_o[:], AF.Identity,
                                             bias=nmr_o, scale=rstd_o)
                        nc.vector.tensor_mul(oh[:], oh[:], gbn_o[:, 0:256])
                        nc.vector.tensor_add(oh[:], oh[:], gbn_o[:, 256:512])
                        eo = spool.tile([128, 256], F32, tag="ex", name="eo")
                        nc.scalar.activation(eo[:], oh[:], AF.Exp)
                        xro = spool.tile([128, 256], F32, tag="xr", name="xro")
                        nc.vector.tensor_scalar_max(xro[:], oh[:], 0.0)
                        nc.vector.scalar_tensor_tensor(xob[:], eo[:], -1.0, xro[:],
                                                       ALU.add, ALU.min)
                    xoT = []
                    for i in range(2):
                        tp = tpool.tile([128, 128], BF16, tag="tp", name=f"tpxo{i}")
                        nc.tensor.transpose(tp[:], xob[:, i * 128:(i + 1) * 128],
                                            eye128b[:])
                        xo_t = spool.tile([128, 128], BF16, tag=f"xoT{i}")
                        nc.vector.tensor_copy(xo_t[:], tp[:])
                        xoT.append(xo_t)

                    # ---- obs dist (feature-major): distT[2S, BL] ----
                    pd = dpool.tile([128, 96], F32, tag="pd")
                    for c in range(8):
                        pos = 0 if c < 4 else 64
                        nc.tensor.matmul(
                            pd[pos:pos + 64, 0:16],
                            Wod[:, c * 64:(c + 1) * 64],
                            xoT[c // 4][:, 32 * (c % 4):32 * (c % 4) + 16],
                            start=(c % 4 == 0), stop=(c % 4 == 3),
                            tile_position=(0, pos))
                    dc0 = spool.tile([64, BL], F32, tag="dc0")
                    dc1 = spool.tile([64, BL], F32, tag="dc1")
                    nc.vector.tensor_copy(dc0[:], pd[0:64, 0:16])
                    nc.scalar.copy(dc1[:], pd[64:128, 0:16])
                    distT = spool.tile([64, BL], F32, tag="distT")
                    nc.vector.tensor_add(distT[:], dc0[:], dc1[:])
                    if not affine_trivial:
                        nc.vector.tensor_add(distT[:], distT[:], bod[:])
                    nc.sync.dma_start(o_meanT.ap()[:, t, :], distT[0:STOCH, :])
                    nc.sync.dma_start(o_rawsT.ap()[:, t, :], distT[STOCH:, :])

                    # ---- next-step in45T + xop prefetch ----
                    if t + 1 < T:
                        in45T = spool2.tile([45, BL], BF16, tag="in45T")
                        nc.vector.tensor_tensor(
                            in45T[0:STOCH, :], distT[0:STOCH, :],
                            maskTb[:, (t + 1) * BL:(t + 2) * BL], ALU.mult)
                        nc.gpsimd.dma_start(in45T[STOCH:, :],
                                            d_am1T.ap()[:, (t + 1) * BL:(t + 2) * BL])
                        xop_f = spool2.tile([128, 256], F32, tag="xop_f")
                        for s in range(4):
                            nc.gpsimd.dma_start(
                                xop_f[32 * s:32 * s + 16, :],
                                s_xop.ap()[(t + 1) * BL:(t + 2) * BL,
                                           s * 256:(s + 1) * 256])
                    if t == T - 1:
                        for (hk_, mt_) in group_ready.get(t, []):
                            emit_phase2_group(hk_, mt_)


    nc.compile()
    return nc


def _fold_rows(a):
    """[16, X] -> [128, X]: batch rows replicated at partitions 32s+b."""
    out = np.zeros((128,) + a.shape[1:], a.dtype)
    for s in range(4):
        out[32 * s:32 * s + 16] = a
    return out


def kernel(embed, action, is_first, ens_index,
           W_img_in, b_img_in, g_img_in, bn_img_in,
           W_gru, b_gru, g_gru, bn_gru,
           W_img_out, b_img_out, g_img_out, bn_img_out,
           W_img_dist, b_img_dist,
           W_obs_out, b_obs_out, g_obs_out, bn_obs_out,
           W_obs_dist, b_obs_dist):
    embed = np.asarray(embed, np.float32)
    action = np.asarray(action, np.float32)
    is_first = np.asarray(is_first)
    ens_index = np.asarray(ens_index, np.int64)

    affine_trivial = bool(
        np.all(np.asarray(b_img_in) == 0) and np.all(np.asarray(g_img_in) == 1)
        and np.all(np.asarray(bn_img_in) == 0)
        and np.all(np.asarray(b_gru) == 0) and np.all(np.asarray(g_gru) == 1)
        and np.all(np.asarray(bn_gru) == 0)
        and np.all(np.asarray(b_img_out) == 0) and np.all(np.asarray(g_img_out) == 1)
        and np.all(np.asarray(bn_img_out) == 0)
        and np.all(np.asarray(b_img_dist) == 0)
        and np.all(np.asarray(b_obs_out) == 0) and np.all(np.asarray(g_obs_out) == 1)
        and np.all(np.asarray(bn_obs_out) == 0)
        and np.all(np.asarray(b_obs_dist) == 0))

    key = (tuple(int(x) for x in ens_index), affine_trivial)
    if key not in _CACHE:
        _CACHE[key] = _build(ens_index, affine_trivial)
    nc = _CACHE[key]

    bf = lambda a: np.ascontiguousarray(np.asarray(a, np.float32)).astype(ml_dtypes.bfloat16)
    f32 = np.float32
    mask = 1.0 - np.asarray(is_first, np.float32)          # [B, T]
    a_m = action * mask[:, :, None]                        # [B, T, ACT]

    # ---- weight rearrangement (shared across cores) ----
    Wg_full = np.asarray(W_gru, f32)                       # [2048, 3072]
    v = Wg_full.reshape(16, 128, 3, 8, 128)                # k, kp, g, c, lf
    Wg_r = np.zeros((128, 16 * 8 * 385), f32)
    for k in range(16):
        for c in range(8):
            blk = v[k, :, :, c, :].reshape(128, 384)       # kp, (g,lf)
            col = (k * 8 + c) * 385
            Wg_r[:, col:col + 384] = blk
            Wg_r[:, col + 384] = blk.sum(1)
    Wd_full = np.asarray(W_obs_out, f32)[:DETER]           # [1024, 1024]
    vd = Wd_full.reshape(8, 128, 8, 128)                   # k, kp, c, lf
    Wd_r = np.zeros((128, 8 * 4 * 256), f32)
    for k in range(8):
        for s in range(4):
            col = (k * 4 + s) * 256
            Wd_r[:, col:col + 128] = vd[k, :, s, :]
            Wd_r[:, col + 128:col + 256] = vd[k, :, s + 4, :]
    Wimg_full = np.concatenate([np.asarray(W_img_in, f32),
                                np.asarray(b_img_in, f32)[None, :]], 0)  # [45, 1024]
    vi = Wimg_full.reshape(45, 8, 128)
    Wimg_r = np.zeros((45, 4 * 257), f32)
    for s in range(4):
        col = s * 257
        blk = np.concatenate([vi[:, s, :], vi[:, s + 4, :]], 1)          # [45, 256]
        Wimg_r[:, col:col + 256] = blk
        Wimg_r[:, col + 256] = blk.sum(1)
    Wod_full = np.asarray(W_obs_dist, f32)                 # [1024, 64]
    Wod_r = np.concatenate([Wod_full[c * 128:(c + 1) * 128, :] for c in range(8)], 1)

    selg = np.zeros((128, 128), f32)
    selh = np.zeros((128, 128), f32)
    for p in range(16):
        for s in range(4):
            for s2 in range(4):
                selg[32 * s + p, 32 * s2 + p] = 1.0 / (3 * DETER)
                selh[32 * s + p, 32 * s2 + p] = 1.0 / HIDDEN

    def fold_cols(vec1024):
        """[1024] -> [128, 256] folded (chunk c at strip c%4, colgrp c//4)."""
        vv = np.asarray(vec1024, f32).reshape(8, 128)
        out = np.zeros((128, 256), f32)
        for c in range(8):
            out[32 * (c % 4):32 * (c % 4) + 16,
                (c // 4) * 128:(c // 4) * 128 + 128] = vv[c]
        return out

    def fold_gru(vec3072):
        """[3072] -> [2, 128, 384] per-bank folded (g,lf cols; chunk 4i+s)."""
        vv = np.asarray(vec3072, f32).reshape(3, 8, 128)   # g, c, lf
        out = np.zeros((2, 128, 384), f32)
        for c in range(8):
            i, s = c // 4, c % 4
            out[i, 32 * s:32 * s + 16, :] = vv[:, c, :].reshape(384)
        return out

    gg, gbn = fold_gru(g_gru), fold_gru(bn_gru)
    gbn_gru = np.zeros((128, 4 * 384), f32)
    for i in range(2):
        gbn_gru[:, (2 * i) * 384:(2 * i + 1) * 384] = gg[i]
        gbn_gru[:, (2 * i + 1) * 384:(2 * i + 2) * 384] = gbn[i]

    # bias_obs column-permuted to match s_xop layout (chunk c -> (c%4)*256+(c//4)*128)
    bo = np.asarray(b_obs_out, f32).reshape(8, 128)
    bias_obs_p = np.zeros((1, HIDDEN), f32)
    for c in range(8):
        bias_obs_p[0, (c % 4) * 256 + (c // 4) * 128:(c % 4) * 256 + (c // 4) * 128 + 128] = bo[c]
    bias_obs_p = np.tile(bias_obs_p, (128, 1))

    rep = {
        "We": bf(np.asarray(W_obs_out, f32)[DETER:]),
        "Wg": bf(Wg_r), "Wd": bf(Wd_r), "Wimg": bf(Wimg_r), "Wod": bf(Wod_r),
        "Wio": bf(W_img_out), "Wids": bf(W_img_dist),
        "selg": selg, "selh": selh,
        "eye128b": np.eye(128, dtype=f32).astype(ml_dtypes.bfloat16),
        "eye128": np.eye(128, dtype=f32),
        "gbn_gru": gbn_gru,
        "gbn_obs": np.concatenate([fold_cols(g_obs_out), fold_cols(bn_obs_out)], 1),
        "gbn_img": np.concatenate([fold_cols(g_img_in), fold_cols(bn_img_in)], 1),
        "gbn_ens": np.ascontiguousarray(np.tile(np.concatenate(
            [np.asarray(g_img_out, f32), np.asarray(bn_img_out, f32)], 1)[:, None, :],
            (1, 128, 1))),
        "bias_obs": bias_obs_p,
        "bias_ens": np.ascontiguousarray(
            np.tile(np.asarray(b_img_out, f32)[:, None, :], (1, 128, 1))),
        "b_od": np.ascontiguousarray(
            np.tile(np.asarray(b_obs_dist, f32)[:, None], (1, BL))),
        "b_ids": np.ascontiguousarray(
            np.tile(np.asarray(b_img_dist, f32)[:, None, :], (1, 128, 1))),
    }
    in_maps = []
    for j in range(NC):
        sl = slice(j * BL, (j + 1) * BL)
        e = embed[sl]                                      # [16, T, EMB]
        embT = np.ascontiguousarray(e.transpose(2, 1, 0).reshape(EMB, T * BL))
        am1 = np.concatenate([a_m[sl], np.ones((BL, T, 1), f32)], -1)
        am1T = np.ascontiguousarray(am1.transpose(2, 1, 0).reshape(ACT + 1, T * BL))
        m_j = np.concatenate([mask[sl], np.ones((BL, 1), f32)], 1)  # [16, T+1]
        maskF = _fold_rows(m_j)                            # [128, T+1]
        maskTb = np.tile(np.ascontiguousarray(m_j.T).reshape(-1)[None, :],
                         (STOCH, 1))                       # [32, (T+1)*16]
        im = dict(rep)
        im["embT"] = bf(embT)
        im["am1T"] = bf(am1T)
        im["maskF"] = np.ascontiguousarray(maskF)
        im["maskTb"] = np.ascontiguousarray(maskTb, dtype=f32)
        in_maps.append(im)

    global _LAST_IN_MAPS
    _LAST_IN_MAPS = in_maps
    res = bass_utils.run_bass_kernel_spmd(nc, in_maps, core_ids=list(range(NC)))

    # ---- host-side unfold/assembly ----
    def softplus(x):
        return np.logaddexp(0.0, x) + MIN_STD

    omean = np.empty((B, T, STOCH), f32)
    ostd = np.empty((B, T, STOCH), f32)
    deter_full = np.empty((B, T, DETER), f32)
    o_ens_full = np.concatenate([res.results[j]["o_ens"] for j in range(NC)], 0)
    for j in range(NC):
        r = res.results[j]
        sl = slice(j * BL, (j + 1) * BL)
        omean[sl] = r["o_meanT"].transpose(2, 1, 0)        # [32, T, 16] -> [16, T, 32]
        ostd[sl] = softplus(r["o_rawsT"].transpose(2, 1, 0))
        dF = r["o_detF"]                                   # [128, T, 256]
        arr = dF.reshape(4, 32, T, 2, 128)[:, :16]         # s, b, t, i, lf
        deter_full[sl] = arr.transpose(1, 2, 3, 0, 4).reshape(BL, T, DETER)

    pmean = o_ens_full[..., :STOCH]
    pstd = softplus(o_ens_full[..., STOCH:])
    out = np.concatenate([omean, ostd, omean, pmean, pstd, pmean, deter_full], -1)
    return np.ascontiguousarray(out, dtype=np.float32)
